# revision 1
# baseline (speedup 1.0000x reference)
import sys

for _p in ("/opt/trn_rl_repo",):
    if _p not in sys.path:
        sys.path.insert(0, _p)

import numpy as np
import ml_dtypes

import concourse.bass as bass
import concourse.bacc as bacc
import concourse.tile as tile
import concourse.mybir as mybir
from concourse import bass_utils

F32 = mybir.dt.float32
BF16 = mybir.dt.bfloat16
FP8 = mybir.dt.float8e4

NP_BF16 = ml_dtypes.bfloat16
NP_FP8 = ml_dtypes.float8_e4m3

EMBED = 512
MID = 512
FINAL = 1024
GLIMPSES = 2
NOBJ = 2048
NREL = 32768
NCORES = 8
RCH = NREL // NCORES          # 4096 relations per core
NOB = NOBJ // 128             # 16 object partition-blocks
CSCALE = 65536.0              # fp8 range scaling for qs

_CACHE = {}


def _wn(v, g):
    return (v * (g / np.linalg.norm(v.astype(np.float64)))).astype(np.float32)


def _build():
    """Builds the Bass program once. Returns (nc, input tensor names)."""
    nc = bacc.Bacc(
        "TRN2",
        target_bir_lowering=False,
        debug=False,
        enable_asserts=False,
        num_devices=NCORES,
    )

    # ---- DRAM I/O -------------------------------------------------------
    d_g8 = nc.dram_tensor("g8", [NOBJ, RCH], FP8, kind="ExternalInput")
    d_oht = nc.dram_tensor("oht", [153, RCH], BF16, kind="ExternalInput")
    d_ohet = nc.dram_tensor("ohet", [151, NOBJ], BF16, kind="ExternalInput")
    d_abc = nc.dram_tensor("abc", [GLIMPSES, 153, MID], BF16, kind="ExternalInput")
    d_objT = nc.dram_tensor("objT", [EMBED, 151], BF16, kind="ExternalInput")
    d_objtab = nc.dram_tensor("objtab", [151, EMBED], BF16, kind="ExternalInput")
    d_cnt2 = nc.dram_tensor("cnt2", [128, 2], BF16, kind="ExternalInput")
    d_wqT = nc.dram_tensor("wqT", [GLIMPSES, EMBED, MID], BF16, kind="ExternalInput")
    d_bq = nc.dram_tensor("bq", [GLIMPSES, MID], BF16, kind="ExternalInput")
    d_waT = nc.dram_tensor("waT", [GLIMPSES, MID, MID], F32, kind="ExternalInput")
    d_baT = nc.dram_tensor("baT", [GLIMPSES, 128, 4], F32, kind="ExternalInput")
    d_fc1T = nc.dram_tensor("fc1T", [MID, MID], F32, kind="ExternalInput")
    d_fc1bT = nc.dram_tensor("fc1bT", [128, 4], F32, kind="ExternalInput")
    d_fc2T = nc.dram_tensor("fc2T", [MID, FINAL], F32, kind="ExternalInput")
    d_fc2b = nc.dram_tensor("fc2b", [1, FINAL], F32, kind="ExternalInput")
    d_ones = nc.dram_tensor("ones", [1, 128], BF16, kind="ExternalInput")
    d_recipC = nc.dram_tensor("recipC", [128, 16], F32, kind="ExternalInput")
    d_out = nc.dram_tensor("out", [1, FINAL], F32, kind="ExternalOutput")

    with tile.TileContext(nc) as tc:
        _emit(nc, tc, locals())

    nc.compile()
    in_names = [
        "g8", "oht", "ohet", "abc", "objT", "objtab", "cnt2", "wqT", "bq",
        "waT", "baT", "fc1T", "fc1bT", "fc2T", "fc2b", "ones", "recipC",
    ]
    return nc, in_names


def _emit(nc, tc, d):
    AT = mybir.ActivationFunctionType
    OP = mybir.AluOpType
    rg = [list(range(NCORES))]

    with (
        tc.tile_pool(name="persist", bufs=1) as pp,
        tc.tile_pool(name="work", bufs=3) as wp,
        tc.tile_pool(name="pt", bufs=3, space="PSUM") as pt,
        tc.tile_pool(name="pv", bufs=2, space="PSUM") as pv,
        tc.tile_pool(name="pc", bufs=2, space="PSUM") as pc,
        tc.tile_pool(name="pd", bufs=1, space="PSUM") as pd,
        tc.tile_pool(name="dram", bufs=1, space="DRAM") as dp,
    ):
        # ---- persistent SBUF tensors & loads ---------------------------
        g8 = []
        for b in range(NOB):
            t = pp.tile([128, RCH], FP8, name=f"g8s{b}", tag=f"g8s{b}")
            g8.append(t)

        oht_hi = pp.tile([128, RCH], BF16, name="oht_hi", tag="oht_hi")
        nc.sync.dma_start(oht_hi[:], d["d_oht"][0:128, :])
        oht_lo = pp.tile([25, RCH], BF16, name="oht_lo", tag="oht_lo")
        nc.sync.dma_start(oht_lo[:], d["d_oht"][128:153, :])

        ohet_hi = pp.tile([128, NOBJ], BF16, name="ohet_hi", tag="ohet_hi")
        nc.sync.dma_start(ohet_hi[:], d["d_ohet"][0:128, :])
        ohet_lo = pp.tile([23, NOBJ], BF16, name="ohet_lo", tag="ohet_lo")
        nc.sync.dma_start(ohet_lo[:], d["d_ohet"][128:151, :])

        abc_hi = pp.tile([128, GLIMPSES * MID], BF16, name="abc_hi", tag="abc_hi")
        abc_lo = pp.tile([25, GLIMPSES * MID], BF16, name="abc_lo", tag="abc_lo")
        for g in range(GLIMPSES):
            nc.sync.dma_start(abc_hi[:, g * MID:(g + 1) * MID], d["d_abc"][g, 0:128, :])
            nc.sync.dma_start(abc_lo[:, g * MID:(g + 1) * MID], d["d_abc"][g, 128:153, :])

        objT = pp.tile([128, 4 * 151], BF16, name="objT", tag="objT")
        for kb in range(4):
            nc.sync.dma_start(objT[:, kb * 151:(kb + 1) * 151],
                              d["d_objT"][kb * 128:(kb + 1) * 128, :])

        objtab_hi = pp.tile([128, EMBED], BF16, name="objtab_hi", tag="objtab_hi")
        nc.sync.dma_start(objtab_hi[:], d["d_objtab"][0:128, :])
        objtab_lo = pp.tile([23, EMBED], BF16, name="objtab_lo", tag="objtab_lo")
        nc.sync.dma_start(objtab_lo[:], d["d_objtab"][128:151, :])

        cnt2 = pp.tile([128, 2], BF16, name="cnt2", tag="cnt2")
        nc.sync.dma_start(cnt2[:], d["d_cnt2"][:, :])

        wqT = pp.tile([128, GLIMPSES * 4 * MID], BF16, name="wqT", tag="wqT")
        for g in range(GLIMPSES):
            for kb in range(4):
                nc.sync.dma_start(
                    wqT[:, (g * 4 + kb) * MID:(g * 4 + kb + 1) * MID],
                    d["d_wqT"][g, kb * 128:(kb + 1) * 128, :])

        recipC = pp.tile([128, NOB], F32, name="recipC", tag="recipC")
        nc.sync.dma_start(recipC[:], d["d_recipC"][:, :])

        bqrow = pp.tile([1, GLIMPSES * MID], BF16, name="bqrow", tag="bqrow")
        for g in range(GLIMPSES):
            nc.sync.dma_start(bqrow[0:1, g * MID:(g + 1) * MID], d["d_bq"][g:g + 1, :])

        for b in range(NOB):
            nc.sync.dma_start(g8[b][:], d["d_g8"][b * 128:(b + 1) * 128, :])

        waT = pp.tile([128, GLIMPSES * 4 * MID], F32, name="waT", tag="waT")
        for g in range(GLIMPSES):
            for kb in range(4):
                nc.sync.dma_start(
                    waT[:, (g * 4 + kb) * MID:(g * 4 + kb + 1) * MID],
                    d["d_waT"][g, kb * 128:(kb + 1) * 128, :])

        baT = pp.tile([128, GLIMPSES * 4], F32, name="baT", tag="baT")
        for g in range(GLIMPSES):
            nc.sync.dma_start(baT[:, g * 4:(g + 1) * 4], d["d_baT"][g, :, :])

        fc1T = pp.tile([128, 4 * MID], F32, name="fc1T", tag="fc1T")
        for kb in range(4):
            nc.sync.dma_start(fc1T[:, kb * MID:(kb + 1) * MID],
                              d["d_fc1T"][kb * 128:(kb + 1) * 128, :])
        fc1bT = pp.tile([128, 4], F32, name="fc1bT", tag="fc1bT")
        nc.sync.dma_start(fc1bT[:], d["d_fc1bT"][:, :])
        fc2T = pp.tile([128, 4 * FINAL], F32, name="fc2T", tag="fc2T")
        for kb in range(4):
            nc.sync.dma_start(fc2T[:, kb * FINAL:(kb + 1) * FINAL],
                              d["d_fc2T"][kb * 128:(kb + 1) * 128, :])
        fc2b = pp.tile([1, FINAL], F32, name="fc2b", tag="fc2b")
        nc.sync.dma_start(fc2b[:], d["d_fc2b"][:, :])
        ones = pp.tile([1, 128], BF16, name="ones", tag="ones")
        nc.sync.dma_start(ones[:], d["d_ones"][:, :])

        # ---- sgq0 = cnt @ obj_tab (column sums of q0), partition layout
        sgq0_ps = pd.tile([128, 4], F32, name="sgq0_ps", tag="smallps")
        for kt in range(4):
            nc.tensor.matmul(sgq0_ps[:, kt:kt + 1],
                             objtab_hi[:, kt * 128:(kt + 1) * 128],
                             cnt2[:, 0:1], start=True, stop=False)
            nc.tensor.matmul(sgq0_ps[:, kt:kt + 1],
                             objtab_lo[:, kt * 128:(kt + 1) * 128],
                             cnt2[0:23, 1:2], start=False, stop=True)
        sgq0 = pp.tile([128, 4], F32, name="sgq0", tag="sgq0")
        nc.vector.tensor_copy(sgq0[:], sgq0_ps[:])

        # ---- per-glimpse state -----------------------------------------
        qs = [pp.tile([128, 2 * MID], FP8, name=f"qs{p}", tag=f"qs{p}") for p in range(8)]
        qrelu = [pp.tile([128, MID], BF16, name=f"qrelu{t}", tag=f"qrelu{t}") for t in range(NOB)]
        hT = [pp.tile([128, 4], F32, name=f"hT{g}", tag=f"hT{g}") for g in range(GLIMPSES)]
        hT_all = [pp.tile([128, 4], F32, name=f"hTa{g}", tag=f"hTa{g}") for g in range(GLIMPSES)]
        ah_sb = [pp.tile([128, 4], F32, name=f"ah{g}", tag=f"ah{g}") for g in range(GLIMPSES)]
        qw = [pp.tile([128, 2 * MID], BF16, name=f"qw{g}", tag=f"qw{g}") for g in range(GLIMPSES)]
        z1bq = pp.tile([1, MID], BF16, name="z1bq", tag="z1bq")
        ah_bf = pp.tile([128, 4], BF16, name="ah_bf", tag="ah_bf")

        def emit_qw(g, bias_row):
            # QW_g = obj_tab @ WqT_g + 1 x bias_row   (two partition chunks)
            for part, (sl_off, psz, ofree) in enumerate(((0, 128, 0), (128, 23, MID))):
                ps = pc.tile([128, MID], F32, name=f"qwps{g}_{part}", tag="qps")
                for kb in range(4):
                    nc.tensor.matmul(
                        ps[0:psz, :],
                        objT[:, kb * 151 + sl_off: kb * 151 + sl_off + psz],
                        wqT[:, (g * 4 + kb) * MID:(g * 4 + kb + 1) * MID],
                        start=(kb == 0), stop=False)
                nc.tensor.matmul(ps[0:psz, :], ones[0:1, 0:psz], bias_row,
                                 start=False, stop=True)
                nc.scalar.copy(qw[g][0:psz, ofree:ofree + MID], ps[0:psz, :])

        def emit_qs(g):
            # q_ = relu(OHE @ QW_g); qs = q_ * recipC   (fp8)
            for ot in range(NOB):
                ps = pc.tile([128, MID], F32, name=f"qps{g}_{ot}", tag="qps")
                nc.tensor.matmul(ps[:], ohet_hi[:, ot * 128:(ot + 1) * 128],
                                 qw[g][:, 0:MID], start=True, stop=False)
                nc.tensor.matmul(ps[:], ohet_lo[:, ot * 128:(ot + 1) * 128],
                                 qw[g][0:23, MID:2 * MID], start=False, stop=True)
                nc.scalar.activation(qrelu[ot][:], ps[:], AT.Relu)
                nc.vector.tensor_scalar(
                    qs[ot // 2][:, (ot % 2) * MID:(ot % 2 + 1) * MID],
                    qrelu[ot][:], recipC[:, ot:ot + 1], None, OP.mult)

        def emit_glimpse(g):
            # h[mt*128+p] partial via t = graph.T @ qs ; h = sum_r v * t
            hpart = pp.tile([128, 32], F32, name=f"hp{g}", tag=f"hp{g}")
            for mt in range(4):
                for rc in range(8):
                    vps = pv.tile([128, 512], F32, name=f"vps{g}{mt}{rc}", tag="vps")
                    nc.tensor.matmul(
                        vps[:],
                        abc_hi[:, g * MID + mt * 128: g * MID + (mt + 1) * 128],
                        oht_hi[:, rc * 512:(rc + 1) * 512], start=True, stop=False)
                    nc.tensor.matmul(
                        vps[:],
                        abc_lo[:, g * MID + mt * 128: g * MID + (mt + 1) * 128],
                        oht_lo[:, rc * 512:(rc + 1) * 512], start=False, stop=True)
                    vch = wp.tile([128, 512], BF16, name=f"vch{g}{mt}{rc}", tag="vch")
                    nc.scalar.activation(vch[:], vps[:], AT.Relu)

                    tps = pt.tile([128, 512], F32, name=f"tps{g}{mt}{rc}", tag="tps")
                    for ob in range(NOB):
                        nc.tensor.matmul(
                            tps[:],
                            qs[ob // 2][:, (ob % 2) * MID + mt * 128:
                                        (ob % 2) * MID + (mt + 1) * 128],
                            g8[ob][:, rc * 512:(rc + 1) * 512],
                            start=(ob == 0), stop=(ob == NOB - 1))
                    scr = wp.tile([128, 512], BF16, name=f"scr{g}{mt}{rc}", tag="scr")
                    nc.vector.tensor_tensor(scr[:], tps[:], vch[:], OP.mult)
                    nc.vector.tensor_reduce(
                        hpart[:, mt * 8 + rc: mt * 8 + rc + 1], scr[:],
                        mybir.AxisListType.X, OP.add)

            for mt in range(4):
                nc.vector.tensor_reduce(hT[g][:, mt:mt + 1],
                                        hpart[:, mt * 8:(mt + 1) * 8],
                                        mybir.AxisListType.X, OP.add)
            nc.vector.tensor_scalar(hT[g][:], hT[g][:], 1.0 / CSCALE, None, OP.mult)

            # AllReduce h partial
            h_in = dp.tile([128, 4], F32, name=f"h_in{g}", tag=f"h_in{g}")
            h_out = dp.tile([128, 4], F32, name=f"h_out{g}", tag=f"h_out{g}",
                            addr_space="Shared")
            nc.sync.dma_start(h_in[:], hT[g][:])
            nc.gpsimd.collective_compute(
                "AllReduce", OP.add, replica_groups=rg,
                ins=[h_in[:].opt()], outs=[h_out[:].opt()])
            nc.sync.dma_start(hT_all[g][:], h_out[:])

            # ahT = WaT_g.T-blocks @ hT + baT  (fp32, partition layout)
            aps = pd.tile([128, 4], F32, name=f"ahps{g}", tag="smallps")
            for mt in range(4):
                for kb in range(4):
                    nc.tensor.matmul(
                        aps[:, mt:mt + 1],
                        waT[:, (g * 4 + kb) * MID + mt * 128:
                            (g * 4 + kb) * MID + (mt + 1) * 128],
                        hT_all[g][:, kb:kb + 1],
                        start=(kb == 0), stop=(kb == 3))
            nc.vector.tensor_tensor(ah_sb[g][:], aps[:], baT[:, g * 4:(g + 1) * 4], OP.add)

        # ================= schedule =====================================
        emit_qw(0, bqrow[0:1, 0:MID])
        emit_qs(0)
        emit_glimpse(0)

        # z1 = ah0 @ WqT1 ; z1bq = z1 + bq1
        nc.scalar.copy(ah_bf[:], ah_sb[0][:])
        zps = pc.tile([1, MID], F32, name="zps", tag="qps")
        for kb in range(4):
            nc.tensor.matmul(zps[:], ah_bf[:, kb:kb + 1],
                             wqT[:, (4 + kb) * MID:(4 + kb + 1) * MID],
                             start=(kb == 0), stop=(kb == 3))
        nc.vector.tensor_tensor(z1bq[:], zps[:], bqrow[0:1, MID:2 * MID], OP.add)

        emit_qw(1, z1bq[0:1, :])
        emit_qs(1)
        emit_glimpse(1)

        # sg = sgq0 + 2048*(ah0+ah1)    (partition layout [128,4])
        sgT = pp.tile([128, 4], F32, name="sgT", tag="sgT")
        nc.vector.tensor_tensor(sgT[:], ah_sb[0][:], ah_sb[1][:], OP.add)
        nc.vector.tensor_scalar(sgT[:], sgT[:], float(NOBJ), None, OP.mult)
        nc.vector.tensor_tensor(sgT[:], sgT[:], sgq0[:], OP.add)

        # fc1: o1T = relu(fc1T.T-blocks @ sgT + fc1bT)
        o1ps = pd.tile([128, 4], F32, name="o1ps", tag="smallps")
        for jt in range(4):
            for kb in range(4):
                nc.tensor.matmul(
                    o1ps[:, jt:jt + 1],
                    fc1T[:, kb * MID + jt * 128: kb * MID + (jt + 1) * 128],
                    sgT[:, kb:kb + 1], start=(kb == 0), stop=(kb == 3))
        o1T = pp.tile([128, 4], F32, name="o1T", tag="o1T")
        for jt in range(4):
            nc.scalar.activation(o1T[:, jt:jt + 1], o1ps[:, jt:jt + 1],
                                 AT.Relu, bias=fc1bT[:, jt:jt + 1])

        # fc2: out = relu(o1 @ fc2T + fc2b)   [1, 1024]
        out_sb = pp.tile([1, FINAL], F32, name="out_sb", tag="out_sb")
        for half in range(2):
            ops_ = pc.tile([1, 512], F32, name=f"ops{half}", tag="qps")
            for kb in range(4):
                nc.tensor.matmul(
                    ops_[:], o1T[:, kb:kb + 1],
                    fc2T[:, kb * FINAL + half * 512: kb * FINAL + half * 512 + 512],
                    start=(kb == 0), stop=(kb == 3))
            nc.vector.tensor_tensor(out_sb[0:1, half * 512:(half + 1) * 512],
                                    ops_[:], fc2b[0:1, half * 512:(half + 1) * 512],
                                    OP.add)
        nc.vector.tensor_scalar(out_sb[:], out_sb[:], 0.0, None, OP.max)
        nc.sync.dma_start(d["d_out"][:, :], out_sb[:])


def _prep_inputs(entities, relations, graph, obj_tab, head_tab, tail_tab, pred_tab,
                 lin_v_v, lin_v_g, lin_v_b, lin_q_v, lin_q_g, lin_q_b,
                 lin_a_v, lin_a_g, lin_a_b, fc1_w, fc1_b, fc2_w, fc2_b):
    ent = np.asarray(entities).astype(np.int64)
    rel = np.asarray(relations).astype(np.int64)
    graph = np.asarray(graph, dtype=np.float32)

    abc = np.zeros((GLIMPSES, 153, MID), np.float32)
    wqT = np.zeros((GLIMPSES, EMBED, MID), np.float32)
    waT = np.zeros((GLIMPSES, MID, MID), np.float32)
    baT = np.zeros((GLIMPSES, 128, 4), np.float32)
    bq = np.zeros((GLIMPSES, MID), np.float32)
    for g in range(GLIMPSES):
        Wv = _wn(np.asarray(lin_v_v[g], np.float32), float(lin_v_g[g]))
        abc[g, 0:51] = head_tab[:51] @ Wv[:, 0:EMBED].T + np.asarray(lin_v_b[g], np.float32)
        abc[g, 51:102] = tail_tab[:51] @ Wv[:, EMBED:2 * EMBED].T
        abc[g, 102:153] = pred_tab[:51] @ Wv[:, 2 * EMBED:3 * EMBED].T
        Wq = _wn(np.asarray(lin_q_v[g], np.float32), float(lin_q_g[g]))
        wqT[g] = Wq.T
        bq[g] = np.asarray(lin_q_b[g], np.float32)
        Wa = _wn(np.asarray(lin_a_v[g], np.float32), float(lin_a_g[g]))
        waT[g] = Wa.T
        baT[g] = np.asarray(lin_a_b[g], np.float32).reshape(4, 128).T

    oht = np.zeros((NCORES, 153, RCH), NP_BF16)
    ar = np.arange(RCH)
    for c in range(NCORES):
        rc = rel[c * RCH:(c + 1) * RCH]
        m = np.zeros((153, RCH), np.float32)
        m[rc[:, 0], ar] = 1.0
        m[rc[:, 1] + 51, ar] = 1.0
        m[rc[:, 2] + 102, ar] = 1.0
        oht[c] = m.astype(NP_BF16)

    ohet = np.zeros((151, NOBJ), np.float32)
    ohet[ent, np.arange(NOBJ)] = 1.0
    cnt = np.bincount(ent, minlength=151).astype(np.float32)
    cnt2 = np.zeros((128, 2), np.float32)
    cnt2[:, 0] = cnt[0:128]
    cnt2[0:23, 1] = cnt[128:151]

    base = {
        "ohet": ohet.astype(NP_BF16),
        "abc": abc.astype(NP_BF16),
        "objT": np.ascontiguousarray(np.asarray(obj_tab, np.float32).T).astype(NP_BF16),
        "objtab": np.asarray(obj_tab, np.float32).astype(NP_BF16),
        "cnt2": cnt2.astype(NP_BF16),
        "wqT": wqT.astype(NP_BF16),
        "bq": bq.astype(NP_BF16),
        "waT": waT,
        "baT": baT,
        "fc1T": np.ascontiguousarray(fc1_w.astype(np.float32).T),
        "fc1bT": np.asarray(fc1_b, np.float32).reshape(4, 128).T.copy(),
        "fc2T": np.ascontiguousarray(fc2_w.astype(np.float32).T),
        "fc2b": np.asarray(fc2_b, np.float32).reshape(1, FINAL),
        "ones": np.ones((1, 128), NP_BF16),
        "recipC": (CSCALE / (graph.sum(axis=1, dtype=np.float32) + 1e-9)).reshape(NOB, 128).T.copy(),
    }
    in_maps = []
    for c in range(NCORES):
        m = dict(base)
        m["g8"] = np.ascontiguousarray(graph[:, c * RCH:(c + 1) * RCH]).astype(NP_FP8)
        m["oht"] = oht[c]
        in_maps.append(m)
    return in_maps


def kernel(**inputs):
    if "nc" not in _CACHE:
        _CACHE["nc"], _CACHE["in_names"] = _build()
    nc = _CACHE["nc"]
    in_maps = _prep_inputs(**inputs)
    res = bass_utils.run_bass_kernel_spmd(nc, in_maps, core_ids=list(range(NCORES)))
    return np.asarray(res.results[0]["out"], np.float32)



# revision 8
# speedup vs baseline: 1.7345x; 1.7345x over previous
import sys

for _p in ("/opt/trn_rl_repo",):
    if _p not in sys.path:
        sys.path.insert(0, _p)

import numpy as np
import ml_dtypes

import concourse.bass as bass
import concourse.bacc as bacc
import concourse.tile as tile
import concourse.mybir as mybir
from concourse import bass_utils

F32 = mybir.dt.float32
BF16 = mybir.dt.bfloat16
FP8 = mybir.dt.float8e4

NP_BF16 = ml_dtypes.bfloat16
NP_FP8 = ml_dtypes.float8_e4m3

EMBED = 512
MID = 512
FINAL = 1024
GLIMPSES = 2
NOBJ = 2048
NREL = 32768
NCORES = 8
RCH = NREL // NCORES          # 4096 relations per core
NOB = NOBJ // 128             # 16 object partition-blocks
VSCALE = float(2 ** 12)      # fp8 scaling for the abc (v) tables
QSCALE = float(2 ** 12)      # fp8 scaling for the qw tables
CSCALE = float(2 ** 24)      # fp8 scaling for qs (atten-normalized q)
HSCALE = 1.0 / (CSCALE * VSCALE)

_CACHE = {}


def _wn(v, g):
    return (v * (g / np.linalg.norm(v.astype(np.float64)))).astype(np.float32)


def _build():
    """Builds the Bass program once. Returns (nc, input tensor names)."""
    nc = bacc.Bacc(
        "TRN2",
        target_bir_lowering=False,
        debug=False,
        enable_asserts=False,
        num_devices=NCORES,
    )

    # ---- DRAM I/O -------------------------------------------------------
    d = {}
    d["d_g8"] = nc.dram_tensor("g8", [NOBJ, RCH], FP8, kind="ExternalInput")
    d["d_oht"] = nc.dram_tensor("oht", [256, RCH], FP8, kind="ExternalInput")
    d["d_abc"] = nc.dram_tensor("abc", [GLIMPSES, 256, MID], FP8, kind="ExternalInput")
    d["d_ohet"] = nc.dram_tensor("ohet", [256, NOBJ], FP8, kind="ExternalInput")
    d["d_qw"] = nc.dram_tensor("qw", [GLIMPSES, 256, MID], FP8, kind="ExternalInput")
    d["d_wq1T"] = nc.dram_tensor("wq1T", [EMBED, MID], BF16, kind="ExternalInput")
    d["d_bq1"] = nc.dram_tensor("bq1", [1, MID], F32, kind="ExternalInput")
    d["d_waT"] = nc.dram_tensor("waT", [GLIMPSES, MID, MID], BF16, kind="ExternalInput")
    d["d_baT"] = nc.dram_tensor("baT", [GLIMPSES, 128, 4], F32, kind="ExternalInput")
    d["d_fc1T"] = nc.dram_tensor("fc1T", [MID, MID], BF16, kind="ExternalInput")
    d["d_fc1bT"] = nc.dram_tensor("fc1bT", [128, 4], F32, kind="ExternalInput")
    d["d_fc2T"] = nc.dram_tensor("fc2T", [MID, FINAL], BF16, kind="ExternalInput")
    d["d_fc2b"] = nc.dram_tensor("fc2b", [1, FINAL], BF16, kind="ExternalInput")
    d["d_recipC"] = nc.dram_tensor("recipC", [128, NOB], F32, kind="ExternalInput")
    d["d_sgq0"] = nc.dram_tensor("sgq0", [128, 4], F32, kind="ExternalInput")
    d["d_out"] = nc.dram_tensor("out", [1, FINAL], F32, kind="ExternalOutput")

    with tile.TileContext(nc) as tc:
        _emit(nc, tc, d)

    nc.compile()
    in_names = [
        "g8", "oht", "abc", "ohet", "qw", "wq1T", "bq1", "waT", "baT",
        "fc1T", "fc1bT", "fc2T", "fc2b", "recipC", "sgq0",
    ]
    return nc, in_names


def _emit(nc, tc, d):
    AT = mybir.ActivationFunctionType
    OP = mybir.AluOpType
    DR = mybir.MatmulPerfMode.DoubleRow
    rg = [list(range(NCORES))]

    with (
        tc.tile_pool(name="persist", bufs=1) as pp,
        tc.tile_pool(name="vchp", bufs=1) as vp,
        tc.tile_pool(name="work", bufs=3) as wp,
        tc.tile_pool(name="pt", bufs=6, space="PSUM") as pt,
        tc.tile_pool(name="pw", bufs=2, space="PSUM") as pw,
        tc.tile_pool(name="dram", bufs=1, space="DRAM") as dp,
    ):
        # ---- persistent SBUF tensors & loads (in dependency order) ------
        abc3 = []
        for g in range(GLIMPSES):
            t = pp.tile([128, 2, MID], FP8, name=f"abc3_{g}", tag=f"abc3_{g}")
            nc.sync.dma_start(t[:, 0, :], d["d_abc"][g, 0:128, :])
            nc.sync.dma_start(t[:, 1, :], d["d_abc"][g, 128:256, :])
            abc3.append(t)

        oht3 = pp.tile([128, 2, RCH], FP8, name="oht3", tag="oht3")
        nc.sync.dma_start(oht3[:, 0, :], d["d_oht"][0:128, :])
        nc.sync.dma_start(oht3[:, 1, :], d["d_oht"][128:256, :])

        ohet3 = pp.tile([128, 2, NOBJ], FP8, name="ohet3", tag="ohet3")
        nc.sync.dma_start(ohet3[:, 0, :], d["d_ohet"][0:128, :])
        nc.sync.dma_start(ohet3[:, 1, :], d["d_ohet"][128:256, :])

        qw3 = []
        for g in range(GLIMPSES):
            t = pp.tile([128, 2, MID], FP8, name=f"qw3_{g}", tag=f"qw3_{g}")
            nc.sync.dma_start(t[:, 0, :], d["d_qw"][g, 0:128, :])
            nc.sync.dma_start(t[:, 1, :], d["d_qw"][g, 128:256, :])
            qw3.append(t)

        recipC = pp.tile([128, NOB], F32, name="recipC", tag="recipC")
        nc.sync.dma_start(recipC[:], d["d_recipC"][:, :])

        # graph blocks: pairs of 128-row blocks for DoubleRow
        g8p = []
        for b in range(8):
            t = pp.tile([128, 2, RCH], FP8, name=f"g8p{b}", tag=f"g8p{b}")
            nc.sync.dma_start(t[:, 0, :], d["d_g8"][(2 * b) * 128:(2 * b + 1) * 128, :])
            nc.sync.dma_start(t[:, 1, :], d["d_g8"][(2 * b + 1) * 128:(2 * b + 2) * 128, :])
            g8p.append(t)

        # late-use weights
        wq1Ts = pp.tile([128, 4 * MID], BF16, name="wq1Ts", tag="wq1Ts")
        for kb in range(4):
            nc.sync.dma_start(wq1Ts[:, kb * MID:(kb + 1) * MID],
                              d["d_wq1T"][kb * 128:(kb + 1) * 128, :])
        bq1s = pp.tile([1, MID], F32, name="bq1s", tag="bq1s")
        nc.sync.dma_start(bq1s[:], d["d_bq1"][:, :])
        waTs = pp.tile([128, GLIMPSES * 4 * MID], BF16, name="waTs", tag="waTs")
        for g in range(GLIMPSES):
            for kb in range(4):
                nc.sync.dma_start(
                    waTs[:, (g * 4 + kb) * MID:(g * 4 + kb + 1) * MID],
                    d["d_waT"][g, kb * 128:(kb + 1) * 128, :])
        baTs = pp.tile([128, GLIMPSES * 4], F32, name="baTs", tag="baTs")
        for g in range(GLIMPSES):
            nc.sync.dma_start(baTs[:, g * 4:(g + 1) * 4], d["d_baT"][g, :, :])
        fc1Ts = pp.tile([128, 4 * MID], BF16, name="fc1Ts", tag="fc1Ts")
        for kb in range(4):
            nc.sync.dma_start(fc1Ts[:, kb * MID:(kb + 1) * MID],
                              d["d_fc1T"][kb * 128:(kb + 1) * 128, :])
        fc1bTs = pp.tile([128, 4], F32, name="fc1bTs", tag="fc1bTs")
        nc.sync.dma_start(fc1bTs[:], d["d_fc1bT"][:, :])
        fc2Ts = pp.tile([128, 4 * FINAL], BF16, name="fc2Ts", tag="fc2Ts")
        for kb in range(4):
            nc.sync.dma_start(fc2Ts[:, kb * FINAL:(kb + 1) * FINAL],
                              d["d_fc2T"][kb * 128:(kb + 1) * 128, :])
        fc2bs = pp.tile([1, FINAL], BF16, name="fc2bs", tag="fc2bs")
        nc.sync.dma_start(fc2bs[:], d["d_fc2b"][:, :])
        sgq0s = pp.tile([128, 4], F32, name="sgq0s", tag="sgq0s")
        nc.sync.dma_start(sgq0s[:], d["d_sgq0"][:, :])

        # ---- per-glimpse state ------------------------------------------
        qs3 = [pp.tile([128, 2, MID], FP8, name=f"qs3_{b}", tag=f"qs3_{b}")
               for b in range(8)]
        hpart = [pp.tile([128, 32], F32, name=f"hpart{g}", tag=f"hpart{g}")
                 for g in range(GLIMPSES)]
        hT = [pp.tile([128, 4], F32, name=f"hT{g}", tag=f"hT{g}")
              for g in range(GLIMPSES)]
        hTa = [pp.tile([128, 4], F32, name=f"hTa{g}", tag=f"hTa{g}")
               for g in range(GLIMPSES)]
        hTab = [pp.tile([128, 4], BF16, name=f"hTab{g}", tag=f"hTab{g}")
                for g in range(GLIMPSES)]
        ah_sb = [pp.tile([128, 4], F32, name=f"ah{g}", tag=f"ah{g}")
                 for g in range(GLIMPSES)]
        ah_bf = pp.tile([128, 4], BF16, name="ah_bf", tag="ah_bf")
        z1bq_sb = pp.tile([1, MID], F32, name="z1bq_sb", tag="z1bq_sb")
        ones1 = pp.tile([1, 1], BF16, name="ones1", tag="ones1")
        nc.vector.memset(ones1[:], 1.0)

        def emit_v(g, mt, rc, engine):
            # vch = relu(abc.T @ oht) chunk [128 m, 512 r]  (scaled by VSCALE)
            vps = pw.tile([128, 512], F32, name=f"vps{g}_{mt}_{rc}", tag="wps")
            nc.tensor.matmul(vps[:],
                             abc3[g][:, :, mt * 128:(mt + 1) * 128],
                             oht3[:, :, rc * 512:(rc + 1) * 512],
                             start=True, stop=True, perf_mode=DR)
            vch = vp.tile([128, 512], BF16, name=f"vch{g}_{mt}_{rc}",
                          tag=f"vch{mt}_{rc}")
            if engine == 0:
                nc.scalar.activation(vch[:], vps[:], AT.Relu)
            else:
                nc.vector.tensor_scalar(vch[:], vps[:], 0.0, None, OP.max)
            return vch

        def emit_qs(g, ot):
            # qs = relu(OHE @ qw) * recipC   (fp8, scaled by CSCALE)
            qps = pw.tile([128, 512], F32, name=f"qps{g}_{ot}", tag="wps")
            nc.tensor.matmul(qps[:],
                             ohet3[:, :, ot * 128:(ot + 1) * 128],
                             qw3[g][:, :, :],
                             start=True, stop=True, perf_mode=DR)
            dst = qs3[ot // 2][:, ot % 2, :]
            if ot % 2 == 0:
                nc.scalar.activation(dst, qps[:], AT.Relu,
                                     scale=recipC[:, ot:ot + 1])
            else:
                nc.vector.tensor_scalar(dst, qps[:], recipC[:, ot:ot + 1],
                                        0.0, OP.mult, OP.max)

        def emit_tps_tile(g, mt, rc, vch):
            # t^T chunk [128 m, 512 r] = sum_ob qs^T @ g8 ; then fused
            # h-partial = sum_r vch * t
            tps = pt.tile([128, 512], F32, name=f"tps{g}_{mt}_{rc}", tag="tps")
            for b in range(8):
                nc.tensor.matmul(
                    tps[:],
                    qs3[b][:, :, mt * 128:(mt + 1) * 128],
                    g8p[b][:, :, rc * 512:(rc + 1) * 512],
                    start=(b == 0), stop=(b == 7), perf_mode=DR)
            scr = wp.tile([128, 512], BF16, name=f"scr{g}_{mt}_{rc}", tag="scr")
            idx = mt * 8 + rc
            nc.vector.tensor_tensor(scr[:], tps[:], vch[:], OP.mult)
            nc.vector.tensor_reduce(hpart[g][:, idx:idx + 1], scr[:],
                                    mybir.AxisListType.X, OP.add)

        def emit_h_allreduce(g):
            for mt in range(4):
                nc.vector.tensor_reduce(hT[g][:, mt:mt + 1],
                                        hpart[g][:, mt * 8:(mt + 1) * 8],
                                        mybir.AxisListType.X, OP.add)
            nc.vector.tensor_scalar(hT[g][:], hT[g][:], HSCALE, None, OP.mult)
            h_in = dp.tile([128, 4], F32, name=f"h_in{g}", tag=f"h_in{g}")
            h_out = dp.tile([128, 4], F32, name=f"h_out{g}", tag=f"h_out{g}",
                            addr_space="Shared")
            nc.sync.dma_start(h_in[:], hT[g][:])
            nc.gpsimd.collective_compute(
                "AllReduce", OP.add, replica_groups=rg,
                ins=[h_in[:].opt()], outs=[h_out[:].opt()])
            nc.sync.dma_start(hTa[g][:], h_out[:])

        def emit_ah(g):
            # ah = WaT.T-blocks @ h + baT  (bf16 matmul, partition layout)
            nc.scalar.copy(hTab[g][:], hTa[g][:])
            aps = pw.tile([128, 4], F32, name=f"ahps{g}", tag="wps")
            for mt in range(4):
                for kb in range(4):
                    nc.tensor.matmul(
                        aps[:, mt:mt + 1],
                        waTs[:, (g * 4 + kb) * MID + mt * 128:
                             (g * 4 + kb) * MID + (mt + 1) * 128],
                        hTab[g][:, kb:kb + 1],
                        start=(kb == 0), stop=(kb == 3))
            nc.vector.tensor_tensor(ah_sb[g][:], aps[:],
                                    baTs[:, g * 4:(g + 1) * 4], OP.add)

        # ================= schedule =====================================
        # glimpse 0 prologue: v0 + qs0 while g8 streams in
        vch0 = {}
        for mt in range(4):
            for rc in range(8):
                vch0[(mt, rc)] = emit_v(0, mt, rc, (mt * 8 + rc) % 2)
        for ot in range(NOB):
            emit_qs(0, ot)

        # glimpse 0 main loop; interleave glimpse-1 v while PE waits on DMA
        vch1 = {}
        v1_jobs = [(mt, rc) for mt in range(4) for rc in range(8)]
        ti = 0
        for mt in range(4):
            for rc in range(8):
                emit_tps_tile(0, mt, rc, vch0[(mt, rc)])
                if ti < len(v1_jobs):
                    m2, r2 = v1_jobs[ti]
                    vch1[(m2, r2)] = emit_v(1, m2, r2, 0)
                ti += 1

        emit_h_allreduce(0)
        emit_ah(0)

        # z1 = ah0 @ Wq1T + bq1 -> row 151 of qw1 (via ones row in ohet)
        nc.scalar.copy(ah_bf[:], ah_sb[0][:])
        zps = pw.tile([1, MID], F32, name="zps", tag="wps")
        for kb in range(4):
            nc.tensor.matmul(zps[:], ah_bf[:, kb:kb + 1],
                             wq1Ts[:, kb * MID:(kb + 1) * MID],
                             start=(kb == 0), stop=(kb == 3))
        nc.vector.tensor_tensor(z1bq_sb[:], zps[:], bq1s[:], OP.add)
        nc.scalar.activation(qw3[1][32:33, 1, :], z1bq_sb[:], AT.Copy,
                             scale=QSCALE)

        # glimpse 1
        for ot in range(NOB):
            emit_qs(1, ot)
        for mt in range(4):
            for rc in range(8):
                emit_tps_tile(1, mt, rc, vch1[(mt, rc)])
        emit_h_allreduce(1)
        emit_ah(1)

        # sg = sgq0 + 2048*(ah0+ah1)    (partition layout [128,4])
        sgT = pp.tile([128, 4], F32, name="sgT", tag="sgT")
        nc.vector.tensor_tensor(sgT[:], ah_sb[0][:], ah_sb[1][:], OP.add)
        nc.vector.tensor_scalar(sgT[:], sgT[:], float(NOBJ), None, OP.mult)
        nc.vector.tensor_tensor(sgT[:], sgT[:], sgq0s[:], OP.add)
        sgTb = pp.tile([128, 4], BF16, name="sgTb", tag="sgTb")
        nc.scalar.copy(sgTb[:], sgT[:])

        # fc1: o1T = relu(fc1T.T-blocks @ sgT + fc1bT)  (bf16)
        o1ps = pw.tile([128, 4], F32, name="o1ps", tag="wps")
        for jt in range(4):
            for kb in range(4):
                nc.tensor.matmul(
                    o1ps[:, jt:jt + 1],
                    fc1Ts[:, kb * MID + jt * 128: kb * MID + (jt + 1) * 128],
                    sgTb[:, kb:kb + 1], start=(kb == 0), stop=(kb == 3))
        o1Tb = pp.tile([128, 4], BF16, name="o1Tb", tag="o1Tb")
        for jt in range(4):
            nc.scalar.activation(o1Tb[:, jt:jt + 1], o1ps[:, jt:jt + 1],
                                 AT.Relu, bias=fc1bTs[:, jt:jt + 1])

        # fc2: out = relu(o1 @ fc2T + fc2b)   [1, 1024]
        out_sb = pp.tile([1, FINAL], F32, name="out_sb", tag="out_sb")
        for half in range(2):
            ops_ = pw.tile([1, 512], F32, name=f"ops{half}", tag="wps")
            for kb in range(4):
                nc.tensor.matmul(
                    ops_[:], o1Tb[:, kb:kb + 1],
                    fc2Ts[:, kb * FINAL + half * 512: kb * FINAL + half * 512 + 512],
                    start=(kb == 0), stop=False)
            nc.tensor.matmul(
                ops_[:], ones1[:],
                fc2bs[0:1, half * 512:(half + 1) * 512],
                start=False, stop=True)
            nc.scalar.activation(out_sb[0:1, half * 512:(half + 1) * 512],
                                 ops_[:], AT.Relu)
        nc.sync.dma_start(d["d_out"][:, :], out_sb[:])


def _prep_inputs(entities, relations, graph, obj_tab, head_tab, tail_tab, pred_tab,
                 lin_v_v, lin_v_g, lin_v_b, lin_q_v, lin_q_g, lin_q_b,
                 lin_a_v, lin_a_g, lin_a_b, fc1_w, fc1_b, fc2_w, fc2_b):
    ent = np.asarray(entities).astype(np.int64)
    rel = np.asarray(relations).astype(np.int64)
    graph = np.asarray(graph, dtype=np.float32)
    obj_tab = np.asarray(obj_tab, np.float32)
    head_tab = np.asarray(head_tab, np.float32)
    tail_tab = np.asarray(tail_tab, np.float32)
    pred_tab = np.asarray(pred_tab, np.float32)

    abc = np.zeros((GLIMPSES, 256, MID), np.float32)
    qw = np.zeros((GLIMPSES, 256, MID), np.float32)
    waT = np.zeros((GLIMPSES, MID, MID), np.float32)
    baT = np.zeros((GLIMPSES, 128, 4), np.float32)
    for g in range(GLIMPSES):
        Wv = _wn(np.asarray(lin_v_v[g], np.float32), float(lin_v_g[g]))
        abc[g, 0:51] = head_tab[:51] @ Wv[:, 0:EMBED].T + np.asarray(lin_v_b[g], np.float32)
        abc[g, 51:102] = tail_tab[:51] @ Wv[:, EMBED:2 * EMBED].T
        abc[g, 102:153] = pred_tab[:51] @ Wv[:, 2 * EMBED:3 * EMBED].T
        Wq = _wn(np.asarray(lin_q_v[g], np.float32), float(lin_q_g[g]))
        qw[g, 0:151] = obj_tab @ Wq.T
        if g == 0:
            qw[0, 0:151] += np.asarray(lin_q_b[0], np.float32)
        Wa = _wn(np.asarray(lin_a_v[g], np.float32), float(lin_a_g[g]))
        waT[g] = Wa.T
        baT[g] = np.asarray(lin_a_b[g], np.float32).reshape(4, 128).T

    wq1T = np.ascontiguousarray(
        _wn(np.asarray(lin_q_v[1], np.float32), float(lin_q_g[1])).T)

    oht = np.zeros((NCORES, 256, RCH), NP_FP8)
    ar = np.arange(RCH)
    for c in range(NCORES):
        rc = rel[c * RCH:(c + 1) * RCH]
        m = np.zeros((256, RCH), np.float32)
        m[rc[:, 0], ar] = 1.0
        m[rc[:, 1] + 51, ar] = 1.0
        m[rc[:, 2] + 102, ar] = 1.0
        oht[c] = m.astype(NP_FP8)

    ohet = np.zeros((256, NOBJ), np.float32)
    ohet[ent, np.arange(NOBJ)] = 1.0
    # ones row at cat 160 (partition 32 of k-subtile 1, ACT-writable):
    # broadcasts the z1bq correction to every object in glimpse 1
    ohet[160, :] = 1.0

    colsum = graph.sum(axis=1, dtype=np.float32) + 1e-9
    recipC = (CSCALE / (colsum * QSCALE)).reshape(NOB, 128).T.copy()

    cnt = np.bincount(ent, minlength=151).astype(np.float32)
    sgq0 = (cnt @ obj_tab).reshape(4, 128).T.copy()

    base = {
        "oht": None,  # per-core
        "abc": (abc * VSCALE).astype(NP_FP8),
        "ohet": ohet.astype(NP_FP8),
        "qw": (qw * QSCALE).astype(NP_FP8),
        "wq1T": wq1T.astype(NP_BF16),
        "bq1": np.asarray(lin_q_b[1], np.float32).reshape(1, MID),
        "waT": waT.astype(NP_BF16),
        "baT": baT,
        "fc1T": np.ascontiguousarray(fc1_w.astype(np.float32).T).astype(NP_BF16),
        "fc1bT": np.asarray(fc1_b, np.float32).reshape(4, 128).T.copy(),
        "fc2T": np.ascontiguousarray(fc2_w.astype(np.float32).T).astype(NP_BF16),
        "fc2b": np.asarray(fc2_b, np.float32).reshape(1, FINAL).astype(NP_BF16),
        "recipC": recipC,
        "sgq0": sgq0,
    }
    in_maps = []
    for c in range(NCORES):
        m = dict(base)
        m["g8"] = np.ascontiguousarray(graph[:, c * RCH:(c + 1) * RCH]).astype(NP_FP8)
        m["oht"] = oht[c]
        in_maps.append(m)
    return in_maps


def kernel(**inputs):
    if "nc" not in _CACHE:
        _CACHE["nc"], _CACHE["in_names"] = _build()
    nc = _CACHE["nc"]
    in_maps = _prep_inputs(**inputs)
    res = bass_utils.run_bass_kernel_spmd(nc, in_maps, core_ids=list(range(NCORES)))
    return np.asarray(res.results[0]["out"], np.float32)


# revision 16
# speedup vs baseline: 1.8265x; 1.0530x over previous
import sys

for _p in ("/opt/trn_rl_repo",):
    if _p not in sys.path:
        sys.path.insert(0, _p)

import numpy as np
import ml_dtypes

import concourse.bass as bass
import concourse.bacc as bacc
import concourse.tile as tile
import concourse.mybir as mybir
from concourse import bass_utils

F32 = mybir.dt.float32
BF16 = mybir.dt.bfloat16
FP8 = mybir.dt.float8e4

NP_BF16 = ml_dtypes.bfloat16
NP_FP8 = ml_dtypes.float8_e4m3

EMBED = 512
MID = 512
FINAL = 1024
GLIMPSES = 2
NOBJ = 2048
NREL = 32768
NCORES = 8
RCH = NREL // NCORES          # 4096 relations per core
NOB = NOBJ // 128             # 16 object partition-blocks
VSCALE = float(2 ** 12)      # fp8 scaling for the abc (v) tables
QSCALE = float(2 ** 12)      # fp8 scaling for the qw tables
CSCALE = float(2 ** 24)      # fp8 scaling for qs (atten-normalized q)
HSCALE = 1.0 / (CSCALE * VSCALE)

_CACHE = {}


def _wn(v, g):
    return (v * (g / np.linalg.norm(v.astype(np.float64)))).astype(np.float32)


def _build():
    """Builds the Bass program once. Returns (nc, input tensor names)."""
    nc = bacc.Bacc(
        "TRN2",
        target_bir_lowering=False,
        debug=False,
        enable_asserts=False,
        num_devices=NCORES,
    )

    # ---- DRAM I/O -------------------------------------------------------
    d = {}
    d["d_g8"] = nc.dram_tensor("g8", [NOBJ, RCH], FP8, kind="ExternalInput")
    d["d_oht"] = nc.dram_tensor("oht", [256, RCH], FP8, kind="ExternalInput")
    d["d_abc"] = nc.dram_tensor("abc", [GLIMPSES, 256, MID], FP8, kind="ExternalInput")
    d["d_ohet"] = nc.dram_tensor("ohet", [256, NOBJ], FP8, kind="ExternalInput")
    d["d_qw"] = nc.dram_tensor("qw", [GLIMPSES, 256, MID], FP8, kind="ExternalInput")
    d["d_k0T"] = nc.dram_tensor("k0T", [MID, MID], BF16, kind="ExternalInput")
    d["d_kb0"] = nc.dram_tensor("kb0", [1, MID], F32, kind="ExternalInput")
    d["d_m0T"] = nc.dram_tensor("m0T", [MID, MID], BF16, kind="ExternalInput")
    d["d_m1T"] = nc.dram_tensor("m1T", [MID, MID], BF16, kind="ExternalInput")
    d["d_u"] = nc.dram_tensor("u", [128, 4], F32, kind="ExternalInput")
    d["d_fc2T"] = nc.dram_tensor("fc2T", [MID, FINAL], BF16, kind="ExternalInput")
    d["d_fc2b"] = nc.dram_tensor("fc2b", [1, FINAL], BF16, kind="ExternalInput")
    d["d_recipC"] = nc.dram_tensor("recipC", [128, NOB], F32, kind="ExternalInput")
    d["d_out"] = nc.dram_tensor("out", [1, FINAL], F32, kind="ExternalOutput")

    with tile.TileContext(nc) as tc:
        _emit(nc, tc, d)

    nc.compile()
    in_names = [
        "g8", "oht", "abc", "ohet", "qw", "k0T", "kb0", "m0T", "m1T", "u",
        "fc2T", "fc2b", "recipC",
    ]
    return nc, in_names


def _emit(nc, tc, d):
    AT = mybir.ActivationFunctionType
    OP = mybir.AluOpType
    DR = mybir.MatmulPerfMode.DoubleRow
    rg = [list(range(NCORES))]

    with (
        tc.tile_pool(name="persist", bufs=1) as pp,
        tc.tile_pool(name="vchp", bufs=1) as vp,
        tc.tile_pool(name="work", bufs=3) as wp,
        tc.tile_pool(name="pt", bufs=6, space="PSUM") as pt,
        tc.tile_pool(name="pw", bufs=2, space="PSUM") as pw,
        tc.tile_pool(name="dram", bufs=1, space="DRAM") as dp,
    ):
        # ---- persistent SBUF tensors & loads (in dependency order) ------
        abc3 = []
        for g in range(GLIMPSES):
            t = pp.tile([128, 2, MID], FP8, name=f"abc3_{g}", tag=f"abc3_{g}")
            nc.sync.dma_start(t[:, 0, :], d["d_abc"][g, 0:128, :])
            nc.sync.dma_start(t[:, 1, :], d["d_abc"][g, 128:256, :])
            abc3.append(t)

        oht3 = pp.tile([128, 2, RCH], FP8, name="oht3", tag="oht3")
        nc.sync.dma_start(oht3[:, 0, :], d["d_oht"][0:128, :])
        nc.sync.dma_start(oht3[:, 1, :], d["d_oht"][128:256, :])

        ohet3 = pp.tile([128, 2, NOBJ], FP8, name="ohet3", tag="ohet3")
        nc.sync.dma_start(ohet3[:, 0, :], d["d_ohet"][0:128, :])
        nc.sync.dma_start(ohet3[:, 1, :], d["d_ohet"][128:256, :])

        qw3 = []
        for g in range(GLIMPSES):
            t = pp.tile([128, 2, MID], FP8, name=f"qw3_{g}", tag=f"qw3_{g}")
            nc.sync.dma_start(t[:, 0, :], d["d_qw"][g, 0:128, :])
            nc.sync.dma_start(t[:, 1, :], d["d_qw"][g, 128:256, :])
            qw3.append(t)

        recipC = pp.tile([128, NOB], F32, name="recipC", tag="recipC")
        nc.sync.dma_start(recipC[:], d["d_recipC"][:, :])

        # graph blocks: pairs of 128-row blocks for DoubleRow
        g8p = []
        for b in range(8):
            t = pp.tile([128, 2, RCH], FP8, name=f"g8p{b}", tag=f"g8p{b}")
            nc.sync.dma_start(t[:, 0, :], d["d_g8"][(2 * b) * 128:(2 * b + 1) * 128, :])
            nc.sync.dma_start(t[:, 1, :], d["d_g8"][(2 * b + 1) * 128:(2 * b + 2) * 128, :])
            g8p.append(t)

        # late-use weights
        k0Ts = pp.tile([128, 4 * MID], BF16, name="k0Ts", tag="k0Ts")
        for kb in range(4):
            nc.sync.dma_start(k0Ts[:, kb * MID:(kb + 1) * MID],
                              d["d_k0T"][kb * 128:(kb + 1) * 128, :])
        kb0s = pp.tile([1, MID], F32, name="kb0s", tag="kb0s")
        nc.sync.dma_start(kb0s[:], d["d_kb0"][:, :])
        m0Ts = pp.tile([128, 4 * MID], BF16, name="m0Ts", tag="m0Ts")
        for kb in range(4):
            nc.sync.dma_start(m0Ts[:, kb * MID:(kb + 1) * MID],
                              d["d_m0T"][kb * 128:(kb + 1) * 128, :])
        m1Ts = pp.tile([128, 4 * MID], BF16, name="m1Ts", tag="m1Ts")
        for kb in range(4):
            nc.sync.dma_start(m1Ts[:, kb * MID:(kb + 1) * MID],
                              d["d_m1T"][kb * 128:(kb + 1) * 128, :])
        us = pp.tile([128, 4], F32, name="us", tag="us")
        nc.sync.dma_start(us[:], d["d_u"][:, :])
        fc2Ts = pp.tile([128, 4 * FINAL], BF16, name="fc2Ts", tag="fc2Ts")
        for kb in range(4):
            nc.sync.dma_start(fc2Ts[:, kb * FINAL:(kb + 1) * FINAL],
                              d["d_fc2T"][kb * 128:(kb + 1) * 128, :])
        fc2bs = pp.tile([1, FINAL], BF16, name="fc2bs", tag="fc2bs")
        nc.sync.dma_start(fc2bs[:], d["d_fc2b"][:, :])

        # ---- per-glimpse state ------------------------------------------
        qs3 = [pp.tile([128, 2, MID], FP8, name=f"qs3_{b}", tag=f"qs3_{b}")
               for b in range(8)]
        hpart = [pp.tile([128, 32], F32, name=f"hpart{g}", tag=f"hpart{g}")
                 for g in range(GLIMPSES)]
        hT = [pp.tile([128, 4], F32, name=f"hT{g}", tag=f"hT{g}")
              for g in range(GLIMPSES)]
        hTa = [pp.tile([128, 4], F32, name=f"hTa{g}", tag=f"hTa{g}")
               for g in range(GLIMPSES)]
        hTab = [pp.tile([128, 4], BF16, name=f"hTab{g}", tag=f"hTab{g}")
                for g in range(GLIMPSES)]
        z1bq_sb = pp.tile([1, MID], F32, name="z1bq_sb", tag="z1bq_sb")
        w_sb = pp.tile([128, 4], F32, name="w_sb", tag="w_sb")
        ones1 = pp.tile([1, 1], BF16, name="ones1", tag="ones1")
        nc.vector.memset(ones1[:], 1.0)

        def emit_v(g, mt, rc, engine):
            # vch = relu(abc.T @ oht) chunk [128 m, 512 r]  (scaled by VSCALE)
            vps = pw.tile([128, 512], F32, name=f"vps{g}_{mt}_{rc}", tag="wps")
            nc.tensor.matmul(vps[:],
                             abc3[g][:, :, mt * 128:(mt + 1) * 128],
                             oht3[:, :, rc * 512:(rc + 1) * 512],
                             start=True, stop=True, perf_mode=DR)
            vch = vp.tile([128, 512], BF16, name=f"vch{g}_{mt}_{rc}",
                          tag=f"vch{mt}_{rc}")
            if engine == 0:
                nc.scalar.activation(vch[:], vps[:], AT.Relu)
            else:
                nc.vector.tensor_scalar(vch[:], vps[:], 0.0, None, OP.max)
            return vch

        def emit_qs(g, ot):
            # qs = relu(OHE @ qw) * recipC   (fp8, scaled by CSCALE)
            qps = pw.tile([128, 512], F32, name=f"qps{g}_{ot}", tag="wps")
            nc.tensor.matmul(qps[:],
                             ohet3[:, :, ot * 128:(ot + 1) * 128],
                             qw3[g][:, :, :],
                             start=True, stop=True, perf_mode=DR)
            dst = qs3[ot // 2][:, ot % 2, :]
            if ot % 2 == 0:
                nc.scalar.activation(dst, qps[:], AT.Relu,
                                     scale=recipC[:, ot:ot + 1])
            else:
                nc.vector.tensor_scalar(dst, qps[:], recipC[:, ot:ot + 1],
                                        0.0, OP.mult, OP.max)

        def emit_tps_tile(g, mt, rc, vch):
            # t^T chunk [128 m, 512 r] = sum_ob qs^T @ g8 ; then fused
            # h-partial = sum_r vch * t
            tps = pt.tile([128, 512], F32, name=f"tps{g}_{mt}_{rc}", tag="tps")
            for b in range(8):
                nc.tensor.matmul(
                    tps[:],
                    qs3[b][:, :, mt * 128:(mt + 1) * 128],
                    g8p[b][:, :, rc * 512:(rc + 1) * 512],
                    start=(b == 0), stop=(b == 7), perf_mode=DR)
            scr = wp.tile([128, 512], BF16, name=f"scr{g}_{mt}_{rc}", tag="scr")
            idx = mt * 8 + rc
            nc.vector.tensor_tensor(scr[:], tps[:], vch[:], OP.mult)
            nc.vector.tensor_reduce(hpart[g][:, idx:idx + 1], scr[:],
                                    mybir.AxisListType.X, OP.add)

        def emit_h_chunk(g, mt):
            # reduce + scale one 128-row chunk of h, then AllReduce it.
            # Chunked so the first collectives launch while later mt-groups
            # still compute (absorbs cross-core skew).
            nc.vector.tensor_reduce(hT[g][:, mt:mt + 1],
                                    hpart[g][:, mt * 8:(mt + 1) * 8],
                                    mybir.AxisListType.X, OP.add)
            nc.vector.tensor_scalar(hT[g][:, mt:mt + 1], hT[g][:, mt:mt + 1],
                                    HSCALE, None, OP.mult)
            h_in = dp.tile([128, 1], F32, name=f"h_in{g}_{mt}", tag=f"h_in{g}_{mt}")
            h_out = dp.tile([128, 1], F32, name=f"h_out{g}_{mt}",
                            tag=f"h_out{g}_{mt}", addr_space="Shared")
            nc.sync.dma_start(h_in[:], hT[g][:, mt:mt + 1])
            nc.gpsimd.collective_compute(
                "AllReduce", OP.add, replica_groups=rg,
                ins=[h_in[:].opt()], outs=[h_out[:].opt()])
            nc.sync.dma_start(hTa[g][:, mt:mt + 1], h_out[:])

        # ================= schedule =====================================
        # glimpse 0 prologue: v0 + qs0 while g8 streams in
        vch0 = {}
        for mt in range(4):
            for rc in range(8):
                vch0[(mt, rc)] = emit_v(0, mt, rc, (mt * 8 + rc) % 2)
        for ot in range(NOB):
            emit_qs(0, ot)

        # glimpse 0 main loop; interleave glimpse-1 v while PE waits on DMA,
        # and launch the AllReduce of each h chunk as its mt-group finishes
        vch1 = {}
        v1_jobs = [(mt, rc) for mt in range(4) for rc in range(8)]
        ti = 0
        for mt in range(4):
            for rc in range(8):
                emit_tps_tile(0, mt, rc, vch0[(mt, rc)])
                if ti < len(v1_jobs):
                    m2, r2 = v1_jobs[ti]
                    vch1[(m2, r2)] = emit_v(1, m2, r2, 0)
                ti += 1
            emit_h_chunk(0, mt)
        nc.scalar.copy(hTab[0][:], hTa[0][:])

        # z1bq = h0 @ K0.T + kb0 -> row 160 of qw1 (via ones row in ohet)
        zps = pw.tile([1, MID], F32, name="zps", tag="wps")
        for kb in range(4):
            nc.tensor.matmul(zps[:], hTab[0][:, kb:kb + 1],
                             k0Ts[:, kb * MID:(kb + 1) * MID],
                             start=(kb == 0), stop=(kb == 3))
        nc.vector.tensor_tensor(z1bq_sb[:], zps[:], kb0s[:], OP.add)
        nc.scalar.activation(qw3[1][32:33, 1, :], z1bq_sb[:], AT.Copy,
                             scale=QSCALE)

        # glimpse 1
        for ot in range(NOB):
            emit_qs(1, ot)

        # z0 = M0 @ h0 ; w = u + 2048*z0   (runs under the tps1 window)
        z0ps = pw.tile([128, 4], F32, name="z0ps", tag="wps")
        for jt in range(4):
            for kb in range(4):
                nc.tensor.matmul(
                    z0ps[:, jt:jt + 1],
                    m0Ts[:, kb * MID + jt * 128: kb * MID + (jt + 1) * 128],
                    hTab[0][:, kb:kb + 1], start=(kb == 0), stop=(kb == 3))
        nc.vector.tensor_scalar(w_sb[:], z0ps[:], float(NOBJ), None, OP.mult)
        nc.vector.tensor_tensor(w_sb[:], w_sb[:], us[:], OP.add)

        for mt in range(4):
            for rc in range(8):
                emit_tps_tile(1, mt, rc, vch1[(mt, rc)])
            emit_h_chunk(1, mt)
        nc.scalar.copy(hTab[1][:], hTa[1][:])

        # tail: o1 = relu(u + 2048*(M0@h0 + M1@h1)) = relu(2048*z1t + w)
        z1ps = pw.tile([128, 4], F32, name="z1ps", tag="wps")
        for jt in range(4):
            for kb in range(4):
                nc.tensor.matmul(
                    z1ps[:, jt:jt + 1],
                    m1Ts[:, kb * MID + jt * 128: kb * MID + (jt + 1) * 128],
                    hTab[1][:, kb:kb + 1], start=(kb == 0), stop=(kb == 3))
        o1Tb = pp.tile([128, 4], BF16, name="o1Tb", tag="o1Tb")
        for jt in range(4):
            nc.scalar.activation(o1Tb[:, jt:jt + 1], z1ps[:, jt:jt + 1],
                                 AT.Relu, bias=w_sb[:, jt:jt + 1],
                                 scale=float(NOBJ))

        # fc2: out = relu(o1 @ fc2T + fc2b)   [1, 1024]
        out_sb = pp.tile([1, FINAL], F32, name="out_sb", tag="out_sb")
        for half in range(2):
            ops_ = pw.tile([1, 512], F32, name=f"ops{half}", tag="wps")
            for kb in range(4):
                nc.tensor.matmul(
                    ops_[:], o1Tb[:, kb:kb + 1],
                    fc2Ts[:, kb * FINAL + half * 512: kb * FINAL + half * 512 + 512],
                    start=(kb == 0), stop=False)
            nc.tensor.matmul(
                ops_[:], ones1[:],
                fc2bs[0:1, half * 512:(half + 1) * 512],
                start=False, stop=True)
            nc.scalar.activation(out_sb[0:1, half * 512:(half + 1) * 512],
                                 ops_[:], AT.Relu)
        nc.sync.dma_start(d["d_out"][:, :], out_sb[:])


def _prep_inputs(entities, relations, graph, obj_tab, head_tab, tail_tab, pred_tab,
                 lin_v_v, lin_v_g, lin_v_b, lin_q_v, lin_q_g, lin_q_b,
                 lin_a_v, lin_a_g, lin_a_b, fc1_w, fc1_b, fc2_w, fc2_b):
    ent = np.asarray(entities).astype(np.int64)
    rel = np.asarray(relations).astype(np.int64)
    graph = np.asarray(graph, dtype=np.float32)
    obj_tab = np.asarray(obj_tab, np.float32)
    head_tab = np.asarray(head_tab, np.float32)
    tail_tab = np.asarray(tail_tab, np.float32)
    pred_tab = np.asarray(pred_tab, np.float32)

    fc1_w = np.asarray(fc1_w, np.float32)
    fc1_b = np.asarray(fc1_b, np.float32)

    abc = np.zeros((GLIMPSES, 256, MID), np.float32)
    qw = np.zeros((GLIMPSES, 256, MID), np.float32)
    Wa = [None, None]
    ba = [None, None]
    for g in range(GLIMPSES):
        Wv = _wn(np.asarray(lin_v_v[g], np.float32), float(lin_v_g[g]))
        abc[g, 0:51] = head_tab[:51] @ Wv[:, 0:EMBED].T + np.asarray(lin_v_b[g], np.float32)
        abc[g, 51:102] = tail_tab[:51] @ Wv[:, EMBED:2 * EMBED].T
        abc[g, 102:153] = pred_tab[:51] @ Wv[:, 2 * EMBED:3 * EMBED].T
        Wq = _wn(np.asarray(lin_q_v[g], np.float32), float(lin_q_g[g]))
        qw[g, 0:151] = obj_tab @ Wq.T
        if g == 0:
            qw[0, 0:151] += np.asarray(lin_q_b[0], np.float32)
        Wa[g] = _wn(np.asarray(lin_a_v[g], np.float32), float(lin_a_g[g]))
        ba[g] = np.asarray(lin_a_b[g], np.float32)

    Wq1 = _wn(np.asarray(lin_q_v[1], np.float32), float(lin_q_g[1]))
    # z1bq = h0 @ (Wq1 @ Wa0).T + (ba0 @ Wq1.T + bq1)
    k0T = np.ascontiguousarray((Wq1 @ Wa[0]).T)
    kb0 = (ba[0] @ Wq1.T + np.asarray(lin_q_b[1], np.float32)).reshape(1, MID)
    # fc1 @ sg = u + 2048*(M0 @ h0 + M1 @ h1)
    m0T = np.ascontiguousarray((fc1_w @ Wa[0]).T)
    m1T = np.ascontiguousarray((fc1_w @ Wa[1]).T)

    oht = np.zeros((NCORES, 256, RCH), NP_FP8)
    ar = np.arange(RCH)
    for c in range(NCORES):
        rc = rel[c * RCH:(c + 1) * RCH]
        m = np.zeros((256, RCH), np.float32)
        m[rc[:, 0], ar] = 1.0
        m[rc[:, 1] + 51, ar] = 1.0
        m[rc[:, 2] + 102, ar] = 1.0
        oht[c] = m.astype(NP_FP8)

    ohet = np.zeros((256, NOBJ), np.float32)
    ohet[ent, np.arange(NOBJ)] = 1.0
    # ones row at cat 160 (partition 32 of k-subtile 1, ACT-writable):
    # broadcasts the z1bq correction to every object in glimpse 1
    ohet[160, :] = 1.0

    colsum = graph.sum(axis=1, dtype=np.float32) + 1e-9
    recipC = (CSCALE / (colsum * QSCALE)).reshape(NOB, 128).T.copy()

    cnt = np.bincount(ent, minlength=151).astype(np.float32)
    sgq0 = cnt @ obj_tab                       # column sums of q0  [512]
    u = (fc1_w @ sgq0 + float(NOBJ) * (fc1_w @ (ba[0] + ba[1])) + fc1_b)
    u = u.reshape(4, 128).T.copy()

    base = {
        "oht": None,  # per-core
        "abc": (abc * VSCALE).astype(NP_FP8),
        "ohet": ohet.astype(NP_FP8),
        "qw": (qw * QSCALE).astype(NP_FP8),
        "k0T": k0T.astype(NP_BF16),
        "kb0": kb0,
        "m0T": m0T.astype(NP_BF16),
        "m1T": m1T.astype(NP_BF16),
        "u": u,
        "fc2T": np.ascontiguousarray(fc2_w.astype(np.float32).T).astype(NP_BF16),
        "fc2b": np.asarray(fc2_b, np.float32).reshape(1, FINAL).astype(NP_BF16),
        "recipC": recipC,
    }
    in_maps = []
    for c in range(NCORES):
        m = dict(base)
        m["g8"] = np.ascontiguousarray(graph[:, c * RCH:(c + 1) * RCH]).astype(NP_FP8)
        m["oht"] = oht[c]
        in_maps.append(m)
    return in_maps


def kernel(**inputs):
    if "nc" not in _CACHE:
        _CACHE["nc"], _CACHE["in_names"] = _build()
    nc = _CACHE["nc"]
    in_maps = _prep_inputs(**inputs)
    res = bass_utils.run_bass_kernel_spmd(nc, in_maps, core_ids=list(range(NCORES)))
    return np.asarray(res.results[0]["out"], np.float32)


# revision 19
# speedup vs baseline: 1.8988x; 1.0396x over previous
import sys

for _p in ("/opt/trn_rl_repo",):
    if _p not in sys.path:
        sys.path.insert(0, _p)

import numpy as np
import ml_dtypes

import concourse.bass as bass
import concourse.bacc as bacc
import concourse.tile as tile
import concourse.mybir as mybir
from concourse import bass_utils

F32 = mybir.dt.float32
BF16 = mybir.dt.bfloat16
FP8 = mybir.dt.float8e4

NP_BF16 = ml_dtypes.bfloat16
NP_FP8 = ml_dtypes.float8_e4m3

EMBED = 512
MID = 512
FINAL = 1024
GLIMPSES = 2
NOBJ = 2048
NREL = 32768
NCORES = 8
RCH = NREL // NCORES          # 4096 relations per core
NOB = NOBJ // 128             # 16 object partition-blocks
VSCALE = float(2 ** 12)      # fp8 scaling for the abc (v) tables
QSCALE = float(2 ** 12)      # fp8 scaling for the qw tables
CSCALE = float(2 ** 24)      # fp8 scaling for qs (atten-normalized q)
HSCALE = 1.0 / (CSCALE * VSCALE)

_CACHE = {}


def _wn(v, g):
    return (v * (g / np.linalg.norm(v.astype(np.float64)))).astype(np.float32)


def _build():
    """Builds the Bass program once. Returns (nc, input tensor names)."""
    nc = bacc.Bacc(
        "TRN2",
        target_bir_lowering=False,
        debug=False,
        enable_asserts=False,
        num_devices=NCORES,
    )

    # ---- DRAM I/O -------------------------------------------------------
    d = {}
    d["d_g8"] = nc.dram_tensor("g8", [NOBJ, RCH], FP8, kind="ExternalInput")
    d["d_oht"] = nc.dram_tensor("oht", [256, RCH], FP8, kind="ExternalInput")
    d["d_abc"] = nc.dram_tensor("abc", [GLIMPSES, 256, MID], FP8, kind="ExternalInput")
    d["d_ohet"] = nc.dram_tensor("ohet", [256, NOBJ], FP8, kind="ExternalInput")
    d["d_qw"] = nc.dram_tensor("qw", [GLIMPSES, 256, MID], FP8, kind="ExternalInput")
    d["d_k0T"] = nc.dram_tensor("k0T", [MID, MID], BF16, kind="ExternalInput")
    d["d_kb0"] = nc.dram_tensor("kb0", [1, MID], F32, kind="ExternalInput")
    d["d_m0T"] = nc.dram_tensor("m0T", [MID, MID], BF16, kind="ExternalInput")
    d["d_m1T"] = nc.dram_tensor("m1T", [MID, MID], BF16, kind="ExternalInput")
    d["d_u"] = nc.dram_tensor("u", [128, 4], F32, kind="ExternalInput")
    d["d_fc2T"] = nc.dram_tensor("fc2T", [MID, FINAL], BF16, kind="ExternalInput")
    d["d_fc2b"] = nc.dram_tensor("fc2b", [1, FINAL], BF16, kind="ExternalInput")
    d["d_recipC"] = nc.dram_tensor("recipC", [128, NOB], F32, kind="ExternalInput")
    d["d_out"] = nc.dram_tensor("out", [1, FINAL], F32, kind="ExternalOutput")

    with tile.TileContext(nc) as tc:
        _emit(nc, tc, d)

    nc.compile()
    in_names = [
        "g8", "oht", "abc", "ohet", "qw", "k0T", "kb0", "m0T", "m1T", "u",
        "fc2T", "fc2b", "recipC",
    ]
    return nc, in_names


def _emit(nc, tc, d):
    AT = mybir.ActivationFunctionType
    OP = mybir.AluOpType
    DR = mybir.MatmulPerfMode.DoubleRow
    rg = [list(range(NCORES))]

    with (
        tc.tile_pool(name="persist", bufs=1) as pp,
        tc.tile_pool(name="vchp", bufs=1) as vp,
        tc.tile_pool(name="work", bufs=3) as wp,
        tc.tile_pool(name="pt", bufs=6, space="PSUM") as pt,
        tc.tile_pool(name="pw", bufs=2, space="PSUM") as pw,
        tc.tile_pool(name="dram", bufs=1, space="DRAM") as dp,
    ):
        # ---- persistent SBUF tensors & loads (in dependency order) ------
        abc3 = []
        for g in range(GLIMPSES):
            t = pp.tile([128, 2, MID], FP8, name=f"abc3_{g}", tag=f"abc3_{g}")
            nc.sync.dma_start(t[:, 0, :], d["d_abc"][g, 0:128, :])
            nc.sync.dma_start(t[:, 1, :], d["d_abc"][g, 128:256, :])
            abc3.append(t)

        oht3 = pp.tile([128, 2, RCH], FP8, name="oht3", tag="oht3")
        nc.sync.dma_start(oht3[:, 0, :], d["d_oht"][0:128, :])
        nc.sync.dma_start(oht3[:, 1, :], d["d_oht"][128:256, :])

        ohet3 = pp.tile([128, 2, NOBJ], FP8, name="ohet3", tag="ohet3")
        nc.sync.dma_start(ohet3[:, 0, :], d["d_ohet"][0:128, :])
        nc.sync.dma_start(ohet3[:, 1, :], d["d_ohet"][128:256, :])

        qw3 = []
        for g in range(GLIMPSES):
            t = pp.tile([128, 2, MID], FP8, name=f"qw3_{g}", tag=f"qw3_{g}")
            nc.sync.dma_start(t[:, 0, :], d["d_qw"][g, 0:128, :])
            nc.sync.dma_start(t[:, 1, :], d["d_qw"][g, 128:256, :])
            qw3.append(t)

        recipC = pp.tile([128, NOB], F32, name="recipC", tag="recipC")
        nc.sync.dma_start(recipC[:], d["d_recipC"][:, :])

        # graph blocks: pairs of 128-row blocks for DoubleRow
        g8p = []
        for b in range(8):
            t = pp.tile([128, 2, RCH], FP8, name=f"g8p{b}", tag=f"g8p{b}")
            nc.sync.dma_start(t[:, 0, :], d["d_g8"][(2 * b) * 128:(2 * b + 1) * 128, :])
            nc.sync.dma_start(t[:, 1, :], d["d_g8"][(2 * b + 1) * 128:(2 * b + 2) * 128, :])
            g8p.append(t)

        # late-use weights
        k0Ts = pp.tile([128, 4 * MID], BF16, name="k0Ts", tag="k0Ts")
        for kb in range(4):
            nc.sync.dma_start(k0Ts[:, kb * MID:(kb + 1) * MID],
                              d["d_k0T"][kb * 128:(kb + 1) * 128, :])
        kb0s = pp.tile([1, MID], F32, name="kb0s", tag="kb0s")
        nc.sync.dma_start(kb0s[:], d["d_kb0"][:, :])
        m0Ts = pp.tile([128, 4 * MID], BF16, name="m0Ts", tag="m0Ts")
        for kb in range(4):
            nc.sync.dma_start(m0Ts[:, kb * MID:(kb + 1) * MID],
                              d["d_m0T"][kb * 128:(kb + 1) * 128, :])
        m1Ts = pp.tile([128, 4 * MID], BF16, name="m1Ts", tag="m1Ts")
        for kb in range(4):
            nc.sync.dma_start(m1Ts[:, kb * MID:(kb + 1) * MID],
                              d["d_m1T"][kb * 128:(kb + 1) * 128, :])
        us = pp.tile([128, 4], F32, name="us", tag="us")
        nc.sync.dma_start(us[:], d["d_u"][:, :])
        fc2Ts = pp.tile([128, 4 * FINAL], BF16, name="fc2Ts", tag="fc2Ts")
        for kb in range(4):
            nc.sync.dma_start(fc2Ts[:, kb * FINAL:(kb + 1) * FINAL],
                              d["d_fc2T"][kb * 128:(kb + 1) * 128, :])
        fc2bs = pp.tile([1, FINAL], BF16, name="fc2bs", tag="fc2bs")
        nc.sync.dma_start(fc2bs[:], d["d_fc2b"][:, :])

        # ---- per-glimpse state ------------------------------------------
        qs3 = [pp.tile([128, 2, MID], FP8, name=f"qs3_{b}", tag=f"qs3_{b}")
               for b in range(8)]
        hpart = [pp.tile([128, 32], F32, name=f"hpart{g}", tag=f"hpart{g}")
                 for g in range(GLIMPSES)]
        hT = [pp.tile([128, 4], F32, name=f"hT{g}", tag=f"hT{g}")
              for g in range(GLIMPSES)]
        hTa = [pp.tile([128, 4], F32, name=f"hTa{g}", tag=f"hTa{g}")
               for g in range(GLIMPSES)]
        hTab = [pp.tile([128, 4], BF16, name=f"hTab{g}", tag=f"hTab{g}")
                for g in range(GLIMPSES)]
        z1bq_sb = pp.tile([1, MID], F32, name="z1bq_sb", tag="z1bq_sb")
        w_sb = pp.tile([128, 4], F32, name="w_sb", tag="w_sb")
        ones1 = pp.tile([1, 1], BF16, name="ones1", tag="ones1")
        nc.vector.memset(ones1[:], 1.0)

        def emit_v(g, mt, rc, engine):
            # vch = relu(abc.T @ oht) chunk [128 m, 512 r]  (scaled by VSCALE)
            vps = pw.tile([128, 512], F32, name=f"vps{g}_{mt}_{rc}", tag="wps")
            nc.tensor.matmul(vps[:],
                             abc3[g][:, :, mt * 128:(mt + 1) * 128],
                             oht3[:, :, rc * 512:(rc + 1) * 512],
                             start=True, stop=True, perf_mode=DR)
            vch = vp.tile([128, 512], BF16, name=f"vch{g}_{mt}_{rc}",
                          tag=f"vch{mt}_{rc}")
            if engine == 0:
                nc.scalar.activation(vch[:], vps[:], AT.Relu)
            else:
                nc.vector.tensor_scalar(vch[:], vps[:], 0.0, None, OP.max)
            return vch

        def emit_qs(g, ot):
            # qs = relu(OHE @ qw) * recipC   (fp8, scaled by CSCALE)
            qps = pw.tile([128, 512], F32, name=f"qps{g}_{ot}", tag="wps")
            nc.tensor.matmul(qps[:],
                             ohet3[:, :, ot * 128:(ot + 1) * 128],
                             qw3[g][:, :, :],
                             start=True, stop=True, perf_mode=DR)
            dst = qs3[ot // 2][:, ot % 2, :]
            if ot % 2 == 0:
                nc.scalar.activation(dst, qps[:], AT.Relu,
                                     scale=recipC[:, ot:ot + 1])
            else:
                nc.vector.tensor_scalar(dst, qps[:], recipC[:, ot:ot + 1],
                                        0.0, OP.mult, OP.max)

        def emit_tps_tile(g, mt, rc, vch):
            # t^T chunk [128 m, 512 r] = sum_ob qs^T @ g8 ; then fused
            # h-partial = sum_r vch * t
            tps = pt.tile([128, 512], F32, name=f"tps{g}_{mt}_{rc}", tag="tps")
            for b in range(8):
                nc.tensor.matmul(
                    tps[:],
                    qs3[b][:, :, mt * 128:(mt + 1) * 128],
                    g8p[b][:, :, rc * 512:(rc + 1) * 512],
                    start=(b == 0), stop=(b == 7), perf_mode=DR)
            scr = wp.tile([128, 512], BF16, name=f"scr{g}_{mt}_{rc}", tag="scr")
            idx = mt * 8 + rc
            nc.vector.tensor_tensor(scr[:], tps[:], vch[:], OP.mult)
            nc.vector.tensor_reduce(hpart[g][:, idx:idx + 1], scr[:],
                                    mybir.AxisListType.X, OP.add)

        def emit_h_reduce(g, mt):
            nc.vector.tensor_reduce(hT[g][:, mt:mt + 1],
                                    hpart[g][:, mt * 8:(mt + 1) * 8],
                                    mybir.AxisListType.X, OP.add)
            nc.vector.tensor_scalar(hT[g][:, mt:mt + 1], hT[g][:, mt:mt + 1],
                                    HSCALE, None, OP.mult)

        def emit_h_allreduce(g):
            h_in = dp.tile([128, 4], F32, name=f"h_in{g}", tag=f"h_in{g}")
            h_out = dp.tile([128, 4], F32, name=f"h_out{g}", tag=f"h_out{g}",
                            addr_space="Shared")
            nc.sync.dma_start(h_in[:], hT[g][:])
            nc.gpsimd.collective_compute(
                "AllReduce", OP.add, replica_groups=rg,
                ins=[h_in[:].opt()], outs=[h_out[:].opt()])
            nc.sync.dma_start(hTa[g][:], h_out[:])

        # ================= schedule =====================================
        # glimpse 0 prologue: v0 + qs0 while g8 streams in
        vch0 = {}
        for mt in range(4):
            for rc in range(8):
                vch0[(mt, rc)] = emit_v(0, mt, rc, (mt * 8 + rc) % 2)
        for ot in range(NOB):
            emit_qs(0, ot)

        # glimpse 0 main loop; interleave glimpse-1 v while PE waits on DMA,
        # and launch the AllReduce of each h chunk as its mt-group finishes
        vch1 = {}
        v1_jobs = [(mt, rc) for mt in range(4) for rc in range(8)]
        ti = 0
        for mt in range(4):
            for rc in range(8):
                emit_tps_tile(0, mt, rc, vch0[(mt, rc)])
                if ti < len(v1_jobs):
                    m2, r2 = v1_jobs[ti]
                    vch1[(m2, r2)] = emit_v(1, m2, r2, 0)
                ti += 1
            emit_h_reduce(0, mt)
        emit_h_allreduce(0)
        nc.scalar.copy(hTab[0][:], hTa[0][:])

        # z1bq = h0 @ K0.T + kb0 -> row 160 of qw1 (via ones row in ohet)
        zps = pw.tile([1, MID], F32, name="zps", tag="wps")
        for kb in range(4):
            nc.tensor.matmul(zps[:], hTab[0][:, kb:kb + 1],
                             k0Ts[:, kb * MID:(kb + 1) * MID],
                             start=(kb == 0), stop=(kb == 3))
        nc.vector.tensor_tensor(z1bq_sb[:], zps[:], kb0s[:], OP.add)
        nc.scalar.activation(qw3[1][32:33, 1, :], z1bq_sb[:], AT.Copy,
                             scale=QSCALE)

        # glimpse 1
        for ot in range(NOB):
            emit_qs(1, ot)

        # z0 = M0 @ h0 ; w = u + 2048*z0   (runs under the tps1 window)
        z0ps = pw.tile([128, 4], F32, name="z0ps", tag="wps")
        for jt in range(4):
            for kb in range(4):
                nc.tensor.matmul(
                    z0ps[:, jt:jt + 1],
                    m0Ts[:, kb * MID + jt * 128: kb * MID + (jt + 1) * 128],
                    hTab[0][:, kb:kb + 1], start=(kb == 0), stop=(kb == 3))
        nc.vector.tensor_scalar(w_sb[:], z0ps[:], float(NOBJ), None, OP.mult)
        nc.vector.tensor_tensor(w_sb[:], w_sb[:], us[:], OP.add)

        for mt in range(4):
            for rc in range(8):
                emit_tps_tile(1, mt, rc, vch1[(mt, rc)])
            emit_h_reduce(1, mt)
        emit_h_allreduce(1)
        nc.scalar.copy(hTab[1][:], hTa[1][:])

        # tail: o1 = relu(u + 2048*(M0@h0 + M1@h1)) = relu(2048*z1t + w)
        z1ps = pw.tile([128, 4], F32, name="z1ps", tag="wps")
        for jt in range(4):
            for kb in range(4):
                nc.tensor.matmul(
                    z1ps[:, jt:jt + 1],
                    m1Ts[:, kb * MID + jt * 128: kb * MID + (jt + 1) * 128],
                    hTab[1][:, kb:kb + 1], start=(kb == 0), stop=(kb == 3))
        o1Tb = pp.tile([128, 4], BF16, name="o1Tb", tag="o1Tb")
        for jt in range(4):
            nc.scalar.activation(o1Tb[:, jt:jt + 1], z1ps[:, jt:jt + 1],
                                 AT.Relu, bias=w_sb[:, jt:jt + 1],
                                 scale=float(NOBJ))

        # fc2: out = relu(o1 @ fc2T + fc2b)   [1, 1024]
        out_sb = pp.tile([1, FINAL], F32, name="out_sb", tag="out_sb")
        for half in range(2):
            ops_ = pw.tile([1, 512], F32, name=f"ops{half}", tag="wps")
            for kb in range(4):
                nc.tensor.matmul(
                    ops_[:], o1Tb[:, kb:kb + 1],
                    fc2Ts[:, kb * FINAL + half * 512: kb * FINAL + half * 512 + 512],
                    start=(kb == 0), stop=False)
            nc.tensor.matmul(
                ops_[:], ones1[:],
                fc2bs[0:1, half * 512:(half + 1) * 512],
                start=False, stop=True)
            nc.scalar.activation(out_sb[0:1, half * 512:(half + 1) * 512],
                                 ops_[:], AT.Relu)
        nc.sync.dma_start(d["d_out"][:, :], out_sb[:])


def _prep_inputs(entities, relations, graph, obj_tab, head_tab, tail_tab, pred_tab,
                 lin_v_v, lin_v_g, lin_v_b, lin_q_v, lin_q_g, lin_q_b,
                 lin_a_v, lin_a_g, lin_a_b, fc1_w, fc1_b, fc2_w, fc2_b):
    ent = np.asarray(entities).astype(np.int64)
    rel = np.asarray(relations).astype(np.int64)
    graph = np.asarray(graph, dtype=np.float32)
    obj_tab = np.asarray(obj_tab, np.float32)
    head_tab = np.asarray(head_tab, np.float32)
    tail_tab = np.asarray(tail_tab, np.float32)
    pred_tab = np.asarray(pred_tab, np.float32)

    fc1_w = np.asarray(fc1_w, np.float32)
    fc1_b = np.asarray(fc1_b, np.float32)

    abc = np.zeros((GLIMPSES, 256, MID), np.float32)
    qw = np.zeros((GLIMPSES, 256, MID), np.float32)
    Wa = [None, None]
    ba = [None, None]
    for g in range(GLIMPSES):
        Wv = _wn(np.asarray(lin_v_v[g], np.float32), float(lin_v_g[g]))
        abc[g, 0:51] = head_tab[:51] @ Wv[:, 0:EMBED].T + np.asarray(lin_v_b[g], np.float32)
        abc[g, 51:102] = tail_tab[:51] @ Wv[:, EMBED:2 * EMBED].T
        abc[g, 102:153] = pred_tab[:51] @ Wv[:, 2 * EMBED:3 * EMBED].T
        Wq = _wn(np.asarray(lin_q_v[g], np.float32), float(lin_q_g[g]))
        qw[g, 0:151] = obj_tab @ Wq.T
        if g == 0:
            qw[0, 0:151] += np.asarray(lin_q_b[0], np.float32)
        Wa[g] = _wn(np.asarray(lin_a_v[g], np.float32), float(lin_a_g[g]))
        ba[g] = np.asarray(lin_a_b[g], np.float32)

    Wq1 = _wn(np.asarray(lin_q_v[1], np.float32), float(lin_q_g[1]))
    # z1bq = h0 @ (Wq1 @ Wa0).T + (ba0 @ Wq1.T + bq1)
    k0T = np.ascontiguousarray((Wq1 @ Wa[0]).T)
    kb0 = (ba[0] @ Wq1.T + np.asarray(lin_q_b[1], np.float32)).reshape(1, MID)
    # fc1 @ sg = u + 2048*(M0 @ h0 + M1 @ h1)
    m0T = np.ascontiguousarray((fc1_w @ Wa[0]).T)
    m1T = np.ascontiguousarray((fc1_w @ Wa[1]).T)

    oht = np.zeros((NCORES, 256, RCH), NP_FP8)
    ar = np.arange(RCH)
    for c in range(NCORES):
        rc = rel[c * RCH:(c + 1) * RCH]
        m = np.zeros((256, RCH), np.float32)
        m[rc[:, 0], ar] = 1.0
        m[rc[:, 1] + 51, ar] = 1.0
        m[rc[:, 2] + 102, ar] = 1.0
        oht[c] = m.astype(NP_FP8)

    ohet = np.zeros((256, NOBJ), np.float32)
    ohet[ent, np.arange(NOBJ)] = 1.0
    # ones row at cat 160 (partition 32 of k-subtile 1, ACT-writable):
    # broadcasts the z1bq correction to every object in glimpse 1
    ohet[160, :] = 1.0

    colsum = graph.sum(axis=1, dtype=np.float32) + 1e-9
    recipC = (CSCALE / (colsum * QSCALE)).reshape(NOB, 128).T.copy()

    cnt = np.bincount(ent, minlength=151).astype(np.float32)
    sgq0 = cnt @ obj_tab                       # column sums of q0  [512]
    u = (fc1_w @ sgq0 + float(NOBJ) * (fc1_w @ (ba[0] + ba[1])) + fc1_b)
    u = u.reshape(4, 128).T.copy()

    base = {
        "oht": None,  # per-core
        "abc": (abc * VSCALE).astype(NP_FP8),
        "ohet": ohet.astype(NP_FP8),
        "qw": (qw * QSCALE).astype(NP_FP8),
        "k0T": k0T.astype(NP_BF16),
        "kb0": kb0,
        "m0T": m0T.astype(NP_BF16),
        "m1T": m1T.astype(NP_BF16),
        "u": u,
        "fc2T": np.ascontiguousarray(fc2_w.astype(np.float32).T).astype(NP_BF16),
        "fc2b": np.asarray(fc2_b, np.float32).reshape(1, FINAL).astype(NP_BF16),
        "recipC": recipC,
    }
    in_maps = []
    for c in range(NCORES):
        m = dict(base)
        m["g8"] = np.ascontiguousarray(graph[:, c * RCH:(c + 1) * RCH]).astype(NP_FP8)
        m["oht"] = oht[c]
        in_maps.append(m)
    return in_maps


def kernel(**inputs):
    if "nc" not in _CACHE:
        _CACHE["nc"], _CACHE["in_names"] = _build()
    nc = _CACHE["nc"]
    in_maps = _prep_inputs(**inputs)
    res = bass_utils.run_bass_kernel_spmd(nc, in_maps, core_ids=list(range(NCORES)))
    return np.asarray(res.results[0]["out"], np.float32)


# revision 20
# speedup vs baseline: 2.1309x; 1.1223x over previous
import sys

for _p in ("/opt/trn_rl_repo",):
    if _p not in sys.path:
        sys.path.insert(0, _p)

import numpy as np
import ml_dtypes

import concourse.bass as bass
import concourse.bacc as bacc
import concourse.tile as tile
import concourse.mybir as mybir
from concourse import bass_utils

F32 = mybir.dt.float32
BF16 = mybir.dt.bfloat16
FP8 = mybir.dt.float8e4

NP_BF16 = ml_dtypes.bfloat16
NP_FP8 = ml_dtypes.float8_e4m3

EMBED = 512
MID = 512
FINAL = 1024
GLIMPSES = 2
NOBJ = 2048
NREL = 32768
NCORES = 8
RCH = NREL // NCORES          # 4096 relations per core
NOB = NOBJ // 128             # 16 object partition-blocks
VSCALE = float(2 ** 12)      # fp8 scaling for the abc (v) tables
QSCALE = float(2 ** 12)      # fp8 scaling for the qw tables
CSCALE = float(2 ** 24)      # fp8 scaling for qs (atten-normalized q)
HSCALE = 1.0 / (CSCALE * VSCALE)

_CACHE = {}


def _wn(v, g):
    return (v * (g / np.linalg.norm(v.astype(np.float64)))).astype(np.float32)


def _build():
    """Builds the Bass program once. Returns (nc, input tensor names)."""
    nc = bacc.Bacc(
        "TRN2",
        target_bir_lowering=False,
        debug=False,
        enable_asserts=False,
        num_devices=NCORES,
    )

    # ---- DRAM I/O -------------------------------------------------------
    d = {}
    d["d_g8"] = nc.dram_tensor("g8", [NOBJ, RCH], FP8, kind="ExternalInput")
    d["d_oht"] = nc.dram_tensor("oht", [256, RCH], FP8, kind="ExternalInput")
    d["d_abc"] = nc.dram_tensor("abc", [GLIMPSES, 256, MID], FP8, kind="ExternalInput")
    d["d_ohet"] = nc.dram_tensor("ohet", [256, NOBJ], FP8, kind="ExternalInput")
    d["d_qw"] = nc.dram_tensor("qw", [GLIMPSES, 256, MID], FP8, kind="ExternalInput")
    d["d_k0T"] = nc.dram_tensor("k0T", [MID, MID], BF16, kind="ExternalInput")
    d["d_kb0"] = nc.dram_tensor("kb0", [1, MID], F32, kind="ExternalInput")
    d["d_m0T"] = nc.dram_tensor("m0T", [MID, MID], BF16, kind="ExternalInput")
    d["d_m1T"] = nc.dram_tensor("m1T", [MID, MID], BF16, kind="ExternalInput")
    d["d_u"] = nc.dram_tensor("u", [128, 4], F32, kind="ExternalInput")
    d["d_fc2T"] = nc.dram_tensor("fc2T", [MID, FINAL], BF16, kind="ExternalInput")
    d["d_fc2b"] = nc.dram_tensor("fc2b", [1, FINAL], BF16, kind="ExternalInput")
    d["d_recipC"] = nc.dram_tensor("recipC", [128, NOB], F32, kind="ExternalInput")
    d["d_out"] = nc.dram_tensor("out", [1, FINAL], F32, kind="ExternalOutput")

    with tile.TileContext(nc) as tc:
        _emit(nc, tc, d)

    nc.compile()
    in_names = [
        "g8", "oht", "abc", "ohet", "qw", "k0T", "kb0", "m0T", "m1T", "u",
        "fc2T", "fc2b", "recipC",
    ]
    return nc, in_names


def _emit(nc, tc, d):
    AT = mybir.ActivationFunctionType
    OP = mybir.AluOpType
    DR = mybir.MatmulPerfMode.DoubleRow
    rg = [list(range(NCORES))]

    with (
        tc.tile_pool(name="persist", bufs=1) as pp,
        tc.tile_pool(name="vchp", bufs=1) as vp,
        tc.tile_pool(name="work", bufs=3) as wp,
        tc.tile_pool(name="pt", bufs=6, space="PSUM") as pt,
        tc.tile_pool(name="pw", bufs=2, space="PSUM") as pw,
        tc.tile_pool(name="dram", bufs=1, space="DRAM") as dp,
    ):
        # ---- persistent SBUF tensors & loads (in dependency order) ------
        abc3 = []
        for g in range(GLIMPSES):
            t = pp.tile([128, 2, MID], FP8, name=f"abc3_{g}", tag=f"abc3_{g}")
            nc.sync.dma_start(t[:, 0, :], d["d_abc"][g, 0:128, :])
            nc.sync.dma_start(t[:, 1, :], d["d_abc"][g, 128:256, :])
            abc3.append(t)

        oht3 = pp.tile([128, 2, RCH], FP8, name="oht3", tag="oht3")
        nc.sync.dma_start(oht3[:, 0, :], d["d_oht"][0:128, :])
        nc.sync.dma_start(oht3[:, 1, :], d["d_oht"][128:256, :])

        ohet3 = pp.tile([128, 2, NOBJ], FP8, name="ohet3", tag="ohet3")
        nc.sync.dma_start(ohet3[:, 0, :], d["d_ohet"][0:128, :])
        nc.sync.dma_start(ohet3[:, 1, :], d["d_ohet"][128:256, :])

        qw3 = []
        for g in range(GLIMPSES):
            t = pp.tile([128, 2, MID], FP8, name=f"qw3_{g}", tag=f"qw3_{g}")
            nc.sync.dma_start(t[:, 0, :], d["d_qw"][g, 0:128, :])
            nc.sync.dma_start(t[:, 1, :], d["d_qw"][g, 128:256, :])
            qw3.append(t)

        recipC = pp.tile([128, NOB], F32, name="recipC", tag="recipC")
        nc.sync.dma_start(recipC[:], d["d_recipC"][:, :])

        # graph blocks: pairs of 128-row blocks for DoubleRow
        g8p = []
        for b in range(8):
            t = pp.tile([128, 2, RCH], FP8, name=f"g8p{b}", tag=f"g8p{b}")
            nc.sync.dma_start(t[:, 0, :], d["d_g8"][(2 * b) * 128:(2 * b + 1) * 128, :])
            nc.sync.dma_start(t[:, 1, :], d["d_g8"][(2 * b + 1) * 128:(2 * b + 2) * 128, :])
            g8p.append(t)

        # late-use weights
        k0Ts = pp.tile([128, 4 * MID], BF16, name="k0Ts", tag="k0Ts")
        for kb in range(4):
            nc.sync.dma_start(k0Ts[:, kb * MID:(kb + 1) * MID],
                              d["d_k0T"][kb * 128:(kb + 1) * 128, :])
        kb0s = pp.tile([1, MID], F32, name="kb0s", tag="kb0s")
        nc.sync.dma_start(kb0s[:], d["d_kb0"][:, :])
        m0Ts = pp.tile([128, 4 * MID], BF16, name="m0Ts", tag="m0Ts")
        for kb in range(4):
            nc.sync.dma_start(m0Ts[:, kb * MID:(kb + 1) * MID],
                              d["d_m0T"][kb * 128:(kb + 1) * 128, :])
        m1Ts = pp.tile([128, 4 * MID], BF16, name="m1Ts", tag="m1Ts")
        for kb in range(4):
            nc.sync.dma_start(m1Ts[:, kb * MID:(kb + 1) * MID],
                              d["d_m1T"][kb * 128:(kb + 1) * 128, :])
        us = pp.tile([128, 4], F32, name="us", tag="us")
        nc.sync.dma_start(us[:], d["d_u"][:, :])
        fc2Ts = pp.tile([128, 4 * FINAL], BF16, name="fc2Ts", tag="fc2Ts")
        for kb in range(4):
            nc.sync.dma_start(fc2Ts[:, kb * FINAL:(kb + 1) * FINAL],
                              d["d_fc2T"][kb * 128:(kb + 1) * 128, :])
        fc2bs = pp.tile([1, FINAL], BF16, name="fc2bs", tag="fc2bs")
        nc.sync.dma_start(fc2bs[:], d["d_fc2b"][:, :])

        # ---- per-glimpse state ------------------------------------------
        qs3 = [pp.tile([128, 2, MID], FP8, name=f"qs3_{b}", tag=f"qs3_{b}")
               for b in range(8)]
        hpart = [pp.tile([128, 32], F32, name=f"hpart{g}", tag=f"hpart{g}")
                 for g in range(GLIMPSES)]
        hT = [pp.tile([128, 4], F32, name=f"hT{g}", tag=f"hT{g}")
              for g in range(GLIMPSES)]
        hTa = [pp.tile([128, 4], F32, name=f"hTa{g}", tag=f"hTa{g}")
               for g in range(GLIMPSES)]
        hTab = [pp.tile([128, 4], BF16, name=f"hTab{g}", tag=f"hTab{g}")
                for g in range(GLIMPSES)]
        z1bq_sb = pp.tile([1, MID], F32, name="z1bq_sb", tag="z1bq_sb")
        w_sb = pp.tile([128, 4], F32, name="w_sb", tag="w_sb")
        ones1 = pp.tile([1, 1], BF16, name="ones1", tag="ones1")
        nc.vector.memset(ones1[:], 1.0)

        def emit_v(g, mt, rc, engine):
            # vch = relu(abc.T @ oht) chunk [128 m, 512 r]  (scaled by VSCALE)
            vps = pw.tile([128, 512], F32, name=f"vps{g}_{mt}_{rc}", tag="wps")
            nc.tensor.matmul(vps[:],
                             abc3[g][:, :, mt * 128:(mt + 1) * 128],
                             oht3[:, :, rc * 512:(rc + 1) * 512],
                             start=True, stop=True, perf_mode=DR)
            vch = vp.tile([128, 512], BF16, name=f"vch{g}_{mt}_{rc}",
                          tag=f"vch{mt}_{rc}")
            if engine == 0:
                nc.scalar.activation(vch[:], vps[:], AT.Relu)
            else:
                nc.vector.tensor_scalar(vch[:], vps[:], 0.0, None, OP.max)
            return vch

        def emit_qs(g, ot):
            # qs = relu(OHE @ qw) * recipC   (fp8, scaled by CSCALE)
            qps = pw.tile([128, 512], F32, name=f"qps{g}_{ot}", tag="wps")
            nc.tensor.matmul(qps[:],
                             ohet3[:, :, ot * 128:(ot + 1) * 128],
                             qw3[g][:, :, :],
                             start=True, stop=True, perf_mode=DR)
            dst = qs3[ot // 2][:, ot % 2, :]
            if ot % 2 == 0:
                nc.scalar.activation(dst, qps[:], AT.Relu,
                                     scale=recipC[:, ot:ot + 1])
            else:
                nc.vector.tensor_scalar(dst, qps[:], recipC[:, ot:ot + 1],
                                        0.0, OP.mult, OP.max)

        def emit_tps_tile(g, mt, rc, vch):
            # t^T chunk [128 m, 512 r] = sum_ob qs^T @ g8 ; then fused
            # h-partial = sum_r vch * t
            tps = pt.tile([128, 512], F32, name=f"tps{g}_{mt}_{rc}", tag="tps")
            for b in range(8):
                nc.tensor.matmul(
                    tps[:],
                    qs3[b][:, :, mt * 128:(mt + 1) * 128],
                    g8p[b][:, :, rc * 512:(rc + 1) * 512],
                    start=(b == 0), stop=(b == 7), perf_mode=DR)
            scr = wp.tile([128, 512], BF16, name=f"scr{g}_{mt}_{rc}", tag="scr")
            idx = mt * 8 + rc
            nc.vector.tensor_tensor(scr[:], tps[:], vch[:], OP.mult)
            nc.vector.tensor_reduce(hpart[g][:, idx:idx + 1], scr[:],
                                    mybir.AxisListType.X, OP.add)

        def emit_h_reduce(g, mt):
            nc.vector.tensor_reduce(hT[g][:, mt:mt + 1],
                                    hpart[g][:, mt * 8:(mt + 1) * 8],
                                    mybir.AxisListType.X, OP.add)
            nc.vector.tensor_scalar(hT[g][:, mt:mt + 1], hT[g][:, mt:mt + 1],
                                    HSCALE, None, OP.mult)

        def emit_h_allreduce(g):
            h_in = dp.tile([128, 4], F32, name=f"h_in{g}", tag=f"h_in{g}")
            h_out = dp.tile([128, 4], F32, name=f"h_out{g}", tag=f"h_out{g}",
                            addr_space="Shared")
            nc.sync.dma_start(h_in[:], hT[g][:])
            nc.gpsimd.collective_compute(
                "AllReduce", OP.add, replica_groups=rg,
                ins=[h_in[:].opt()], outs=[h_out[:].opt()])
            nc.sync.dma_start(hTa[g][:], h_out[:])

        # ================= schedule =====================================
        # glimpse 0 prologue: v0 + qs0 while g8 streams in
        vch0 = {}
        for mt in range(4):
            for rc in range(8):
                vch0[(mt, rc)] = emit_v(0, mt, rc, (mt * 8 + rc) % 2)
        for ot in range(NOB):
            emit_qs(0, ot)

        # glimpse 0 main loop
        for mt in range(4):
            for rc in range(8):
                emit_tps_tile(0, mt, rc, vch0[(mt, rc)])
            emit_h_reduce(0, mt)
        emit_h_allreduce(0)

        # glimpse-1 v runs in the shadow of the AllReduce wait
        vch1 = {}
        for mt in range(4):
            for rc in range(8):
                vch1[(mt, rc)] = emit_v(1, mt, rc, (mt * 8 + rc) % 2)

        nc.scalar.copy(hTab[0][:], hTa[0][:])

        # z1bq = h0 @ K0.T + kb0 -> row 160 of qw1 (via ones row in ohet)
        zps = pw.tile([1, MID], F32, name="zps", tag="wps")
        for kb in range(4):
            nc.tensor.matmul(zps[:], hTab[0][:, kb:kb + 1],
                             k0Ts[:, kb * MID:(kb + 1) * MID],
                             start=(kb == 0), stop=(kb == 3))
        nc.vector.tensor_tensor(z1bq_sb[:], zps[:], kb0s[:], OP.add)
        nc.scalar.activation(qw3[1][32:33, 1, :], z1bq_sb[:], AT.Copy,
                             scale=QSCALE)

        # glimpse 1
        for ot in range(NOB):
            emit_qs(1, ot)

        # z0 = M0 @ h0 ; w = u + 2048*z0   (runs under the tps1 window)
        z0ps = pw.tile([128, 4], F32, name="z0ps", tag="wps")
        for jt in range(4):
            for kb in range(4):
                nc.tensor.matmul(
                    z0ps[:, jt:jt + 1],
                    m0Ts[:, kb * MID + jt * 128: kb * MID + (jt + 1) * 128],
                    hTab[0][:, kb:kb + 1], start=(kb == 0), stop=(kb == 3))
        nc.vector.tensor_scalar(w_sb[:], z0ps[:], float(NOBJ), None, OP.mult)
        nc.vector.tensor_tensor(w_sb[:], w_sb[:], us[:], OP.add)

        for mt in range(4):
            for rc in range(8):
                emit_tps_tile(1, mt, rc, vch1[(mt, rc)])
            emit_h_reduce(1, mt)
        emit_h_allreduce(1)
        nc.scalar.copy(hTab[1][:], hTa[1][:])

        # tail: o1 = relu(u + 2048*(M0@h0 + M1@h1)) = relu(2048*z1t + w)
        z1ps = pw.tile([128, 4], F32, name="z1ps", tag="wps")
        for jt in range(4):
            for kb in range(4):
                nc.tensor.matmul(
                    z1ps[:, jt:jt + 1],
                    m1Ts[:, kb * MID + jt * 128: kb * MID + (jt + 1) * 128],
                    hTab[1][:, kb:kb + 1], start=(kb == 0), stop=(kb == 3))
        o1Tb = pp.tile([128, 4], BF16, name="o1Tb", tag="o1Tb")
        for jt in range(4):
            nc.scalar.activation(o1Tb[:, jt:jt + 1], z1ps[:, jt:jt + 1],
                                 AT.Relu, bias=w_sb[:, jt:jt + 1],
                                 scale=float(NOBJ))

        # fc2: out = relu(o1 @ fc2T + fc2b)   [1, 1024]
        out_sb = pp.tile([1, FINAL], F32, name="out_sb", tag="out_sb")
        for half in range(2):
            ops_ = pw.tile([1, 512], F32, name=f"ops{half}", tag="wps")
            for kb in range(4):
                nc.tensor.matmul(
                    ops_[:], o1Tb[:, kb:kb + 1],
                    fc2Ts[:, kb * FINAL + half * 512: kb * FINAL + half * 512 + 512],
                    start=(kb == 0), stop=False)
            nc.tensor.matmul(
                ops_[:], ones1[:],
                fc2bs[0:1, half * 512:(half + 1) * 512],
                start=False, stop=True)
            nc.scalar.activation(out_sb[0:1, half * 512:(half + 1) * 512],
                                 ops_[:], AT.Relu)
        nc.sync.dma_start(d["d_out"][:, :], out_sb[:])


def _prep_inputs(entities, relations, graph, obj_tab, head_tab, tail_tab, pred_tab,
                 lin_v_v, lin_v_g, lin_v_b, lin_q_v, lin_q_g, lin_q_b,
                 lin_a_v, lin_a_g, lin_a_b, fc1_w, fc1_b, fc2_w, fc2_b):
    ent = np.asarray(entities).astype(np.int64)
    rel = np.asarray(relations).astype(np.int64)
    graph = np.asarray(graph, dtype=np.float32)
    obj_tab = np.asarray(obj_tab, np.float32)
    head_tab = np.asarray(head_tab, np.float32)
    tail_tab = np.asarray(tail_tab, np.float32)
    pred_tab = np.asarray(pred_tab, np.float32)

    fc1_w = np.asarray(fc1_w, np.float32)
    fc1_b = np.asarray(fc1_b, np.float32)

    abc = np.zeros((GLIMPSES, 256, MID), np.float32)
    qw = np.zeros((GLIMPSES, 256, MID), np.float32)
    Wa = [None, None]
    ba = [None, None]
    for g in range(GLIMPSES):
        Wv = _wn(np.asarray(lin_v_v[g], np.float32), float(lin_v_g[g]))
        abc[g, 0:51] = head_tab[:51] @ Wv[:, 0:EMBED].T + np.asarray(lin_v_b[g], np.float32)
        abc[g, 51:102] = tail_tab[:51] @ Wv[:, EMBED:2 * EMBED].T
        abc[g, 102:153] = pred_tab[:51] @ Wv[:, 2 * EMBED:3 * EMBED].T
        Wq = _wn(np.asarray(lin_q_v[g], np.float32), float(lin_q_g[g]))
        qw[g, 0:151] = obj_tab @ Wq.T
        if g == 0:
            qw[0, 0:151] += np.asarray(lin_q_b[0], np.float32)
        Wa[g] = _wn(np.asarray(lin_a_v[g], np.float32), float(lin_a_g[g]))
        ba[g] = np.asarray(lin_a_b[g], np.float32)

    Wq1 = _wn(np.asarray(lin_q_v[1], np.float32), float(lin_q_g[1]))
    # z1bq = h0 @ (Wq1 @ Wa0).T + (ba0 @ Wq1.T + bq1)
    k0T = np.ascontiguousarray((Wq1 @ Wa[0]).T)
    kb0 = (ba[0] @ Wq1.T + np.asarray(lin_q_b[1], np.float32)).reshape(1, MID)
    # fc1 @ sg = u + 2048*(M0 @ h0 + M1 @ h1)
    m0T = np.ascontiguousarray((fc1_w @ Wa[0]).T)
    m1T = np.ascontiguousarray((fc1_w @ Wa[1]).T)

    oht = np.zeros((NCORES, 256, RCH), NP_FP8)
    ar = np.arange(RCH)
    for c in range(NCORES):
        rc = rel[c * RCH:(c + 1) * RCH]
        m = np.zeros((256, RCH), np.float32)
        m[rc[:, 0], ar] = 1.0
        m[rc[:, 1] + 51, ar] = 1.0
        m[rc[:, 2] + 102, ar] = 1.0
        oht[c] = m.astype(NP_FP8)

    ohet = np.zeros((256, NOBJ), np.float32)
    ohet[ent, np.arange(NOBJ)] = 1.0
    # ones row at cat 160 (partition 32 of k-subtile 1, ACT-writable):
    # broadcasts the z1bq correction to every object in glimpse 1
    ohet[160, :] = 1.0

    colsum = graph.sum(axis=1, dtype=np.float32) + 1e-9
    recipC = (CSCALE / (colsum * QSCALE)).reshape(NOB, 128).T.copy()

    cnt = np.bincount(ent, minlength=151).astype(np.float32)
    sgq0 = cnt @ obj_tab                       # column sums of q0  [512]
    u = (fc1_w @ sgq0 + float(NOBJ) * (fc1_w @ (ba[0] + ba[1])) + fc1_b)
    u = u.reshape(4, 128).T.copy()

    base = {
        "oht": None,  # per-core
        "abc": (abc * VSCALE).astype(NP_FP8),
        "ohet": ohet.astype(NP_FP8),
        "qw": (qw * QSCALE).astype(NP_FP8),
        "k0T": k0T.astype(NP_BF16),
        "kb0": kb0,
        "m0T": m0T.astype(NP_BF16),
        "m1T": m1T.astype(NP_BF16),
        "u": u,
        "fc2T": np.ascontiguousarray(fc2_w.astype(np.float32).T).astype(NP_BF16),
        "fc2b": np.asarray(fc2_b, np.float32).reshape(1, FINAL).astype(NP_BF16),
        "recipC": recipC,
    }
    in_maps = []
    for c in range(NCORES):
        m = dict(base)
        m["g8"] = np.ascontiguousarray(graph[:, c * RCH:(c + 1) * RCH]).astype(NP_FP8)
        m["oht"] = oht[c]
        in_maps.append(m)
    return in_maps


def kernel(**inputs):
    if "nc" not in _CACHE:
        _CACHE["nc"], _CACHE["in_names"] = _build()
    nc = _CACHE["nc"]
    in_maps = _prep_inputs(**inputs)
    res = bass_utils.run_bass_kernel_spmd(nc, in_maps, core_ids=list(range(NCORES)))
    return np.asarray(res.results[0]["out"], np.float32)
